# revision 1
# baseline (speedup 1.0000x reference)
"""Trainium2 Bass kernel for a single attention head with input projections.

Per-batch-element (B=8 -> one NeuronCore each):
  k = key @ Wk^T, q = query @ Wq^T, v = value @ Wv^T          [T, H]
  S = q @ k^T / sqrt(E); mask kidx <= qidx+1; P = softmax(S)
  out = P @ v                                                  [T, H]

T=2048, E=1024, H=2048.

Layout strategy: all matmuls contract over the partition dim, so the host
pre-transposes activations and weights to [E, T] / [E, H] (and casts to
bf16).  Scores are computed TRANSPOSED (S^T[tk, tq] = k-tiles as lhsT,
q-tiles as rhs) so that:
  - P^T tiles feed the P@V matmul directly as lhsT (no on-chip transpose),
  - the softmax denominator is a ones-vector matmul over the partition dim.
No max-subtraction is needed: |S| <= ~10 here, exp() is safe in fp32.
"""

import sys

sys.path.insert(0, "/opt/trn_rl_repo")

import ml_dtypes
import numpy as np

import concourse.bass as bass
import concourse.mybir as mybir
import concourse.tile as tile
from concourse import bass_utils
from concourse.tile import ScopedClock

B, T, E, H = 8, 2048, 1024, 2048
P = 128
EO = E // P          # 8 e-subtiles
HO = H // P          # 16 h-subtiles
TKT = T // P         # 16 tk tiles
NBLK = 4             # tq blocks of 512
BLK = T // NBLK      # 512
NMASK = 5            # distinct partial-mask patterns per tq block
BF16 = mybir.dt.bfloat16
F32 = mybir.dt.float32


class _SplitDrainTC(tile.TileContext):
    """This walrus build rejects >1 sync-wait on the kernel-tail SP Drain
    ("Too many sync wait commands").  Spread the waits over preceding nops
    on the same engine instead — sequentially equivalent."""

    def _drain_and_barrier(self, tick_clock, wait_clock):
        nc = self.nc
        nops = [nc.sync.nop(nofuse=True) for _ in range(40)]
        drain_inst = nc.sync.drain()
        wait_clock.add_sem_waits(
            drain_inst.ins, ScopedClock({None: tick_clock.global_clock})
        )
        si = drain_inst.ins.sync_info
        waits = list(si.on_wait or [])
        if len(waits) > 1:
            assert len(waits) <= len(nops) + 1
            si.on_wait = [waits[-1]]
            for w, nop in zip(waits[:-1], nops):
                nsi = nop.ins.sync_info
                if nsi is None:
                    nop.ins.sync_info = mybir.SyncInfo(on_wait=[w], on_update=[])
                else:
                    nsi.on_wait = [w]
        nc.all_engine_barrier()
        popped = nc._tile_sem_poison_stack.pop()
        assert popped is self._sem_poison
        nc.clear_and_free_semaphores(list(self.sems.allocated().values()))
        nc.all_engine_barrier()


def _build():
    nc = bass.Bass("TRN2", target_bir_lowering=False, debug=False)

    xq = nc.dram_tensor("xq", (E, T), BF16, kind="ExternalInput").ap()
    xk = nc.dram_tensor("xk", (E, T), BF16, kind="ExternalInput").ap()
    xv = nc.dram_tensor("xv", (E, T), BF16, kind="ExternalInput").ap()
    wq = nc.dram_tensor("wq", (E, H), BF16, kind="ExternalInput").ap()
    wk = nc.dram_tensor("wk", (E, H), BF16, kind="ExternalInput").ap()
    wv = nc.dram_tensor("wv", (E, H), BF16, kind="ExternalInput").ap()
    masks = nc.dram_tensor("masks", (P, 2 * BLK), BF16, kind="ExternalInput").ap()
    out = nc.dram_tensor("out", (T, H), F32, kind="ExternalOutput").ap()

    def et(a):  # [E, X] dram -> [128, EO, X] view
        return a.rearrange("(eo p) t -> p eo t", p=P)

    with _SplitDrainTC(nc) as tc:
        with (
            tc.tile_pool(name="wkv", bufs=1) as wkv_pool,
            tc.tile_pool(name="wqp", bufs=1) as wq_pool,
            tc.tile_pool(name="xblk", bufs=2) as x_pool,
            tc.tile_pool(name="ktres", bufs=1) as kt_pool,
            tc.tile_pool(name="qt", bufs=1) as qt_pool,
            tc.tile_pool(name="pt", bufs=1) as pt_pool,
            tc.tile_pool(name="vst", bufs=16) as v_pool,
            tc.tile_pool(name="vpj", bufs=1) as vproj_pool,
            tc.tile_pool(name="outs", bufs=2) as out_pool,
            tc.tile_pool(name="misc", bufs=1) as misc_pool,
            tc.tile_pool(name="ps_a", bufs=4, space="PSUM") as ps_a,
            tc.tile_pool(name="ps_o", bufs=3, space="PSUM") as ps_o,
            tc.tile_pool(name="ps_d", bufs=1, space="PSUM") as ps_d,
            tc.tile_pool(name="dram", bufs=1, space="DRAM") as dram_pool,
        ):
            masks_sb = misc_pool.tile([P, 2 * BLK], BF16, tag="masks")
            nc.sync.dma_start(masks_sb[:], masks)
            ones_sb = misc_pool.tile([P, 1], BF16, tag="ones")
            nc.vector.memset(ones_sb[:], 1.0)

            v_dram = dram_pool.tile([T, H], BF16)

            # ---- Phase A1: kT = (Wk xk)  resident in SBUF as [128, HO, T]
            kt_sb = kt_pool.tile([P, HO, T], BF16)
            wk_sb = wkv_pool.tile([P, EO, H], BF16, tag="w")
            for c in range(NBLK):
                nc.sync.dma_start(
                    wk_sb[:, :, c * BLK : (c + 1) * BLK],
                    et(wk)[:, :, c * BLK : (c + 1) * BLK],
                )
            for tb in range(NBLK):
                xk_sb = x_pool.tile([P, EO, BLK], BF16, tag="x")
                nc.sync.dma_start(xk_sb[:], et(xk)[:, :, tb * BLK : (tb + 1) * BLK])
                for ho in range(HO):
                    ps = ps_a.tile([P, BLK], F32, tag="ps_a")
                    for eo in range(EO):
                        nc.tensor.matmul(
                            ps[:],
                            wk_sb[:, eo, ho * P : (ho + 1) * P],
                            xk_sb[:, eo, :],
                            start=(eo == 0),
                            stop=(eo == EO - 1),
                        )
                    nc.vector.tensor_copy(
                        kt_sb[:, ho, tb * BLK : (tb + 1) * BLK], ps[:]
                    )

            # ---- Phase A2: v = (xv^T Wv) -> DRAM scratch [T, H] bf16
            wv_sb = wkv_pool.tile([P, EO, H], BF16, tag="w")
            for c in range(NBLK):
                nc.sync.dma_start(
                    wv_sb[:, :, c * BLK : (c + 1) * BLK],
                    et(wv)[:, :, c * BLK : (c + 1) * BLK],
                )
            # preload wq early so phase B doesn't stall on it
            wq_sb = wq_pool.tile([P, EO, H], BF16, tag="wq")
            nc.sync.dma_start(wq_sb[:], et(wq))
            for tt in range(TKT):
                xv_sb = x_pool.tile([P, EO, P], BF16, tag="xv")
                nc.sync.dma_start(xv_sb[:], et(xv)[:, :, tt * P : (tt + 1) * P])
                v_sb = vproj_pool.tile([P, H], BF16, tag="vproj")
                for hb in range(NBLK):
                    ps = ps_a.tile([P, BLK], F32, tag="ps_a")
                    for eo in range(EO):
                        nc.tensor.matmul(
                            ps[:],
                            xv_sb[:, eo, :],
                            wv_sb[:, eo, hb * BLK : (hb + 1) * BLK],
                            start=(eo == 0),
                            stop=(eo == EO - 1),
                        )
                    nc.vector.tensor_copy(v_sb[:, hb * BLK : (hb + 1) * BLK], ps[:])
                nc.sync.dma_start(v_dram[tt * P : (tt + 1) * P, :], v_sb[:])

            # ---- Phase B: per tq block of 512
            for j in range(NBLK):
                ntk = min(4 * j + 5, TKT)  # tk tiles (mask kidx <= qidx+1)

                xq_sb = x_pool.tile([P, EO, BLK], BF16, tag="x")
                nc.sync.dma_start(xq_sb[:], et(xq)[:, :, j * BLK : (j + 1) * BLK])

                # qT block [128, HO, 512]
                qt_sb = qt_pool.tile([P, HO, BLK], BF16)
                for ho in range(HO):
                    ps = ps_a.tile([P, BLK], F32, tag="ps_a")
                    for eo in range(EO):
                        nc.tensor.matmul(
                            ps[:],
                            wq_sb[:, eo, ho * P : (ho + 1) * P],
                            xq_sb[:, eo, :],
                            start=(eo == 0),
                            stop=(eo == EO - 1),
                        )
                    nc.vector.tensor_copy(qt_sb[:, ho, :], ps[:])

                # S^T tiles -> exp -> mask -> P^T  [128, ntk, 512] bf16
                pt_sb = pt_pool.tile([P, 4 * NBLK, BLK], BF16)
                for t in range(ntk):
                    ps = ps_a.tile([P, BLK], F32, tag="ps_a")
                    for ho in range(HO):
                        nc.tensor.matmul(
                            ps[:],
                            kt_sb[:, ho, t * P : (t + 1) * P],
                            qt_sb[:, ho, :],
                            start=(ho == 0),
                            stop=(ho == HO - 1),
                        )
                    nc.scalar.activation(
                        pt_sb[:, t, :],
                        ps[:],
                        mybir.ActivationFunctionType.Exp,
                        scale=float(E) ** -0.5,
                    )
                    m = t - 4 * j
                    if m >= 0:  # partial tile: zero the disallowed region
                        nc.vector.tensor_tensor(
                            pt_sb[:, t, :],
                            pt_sb[:, t, :],
                            masks_sb[:, BLK - m * P : 2 * BLK - m * P],
                            mybir.AluOpType.mult,
                        )

                # denominator: den[tq] = sum_tk P^T ; ones-matmul, [128, 4]
                den_ps = ps_d.tile([P, NBLK], F32)
                for s in range(NBLK):
                    for t in range(ntk):
                        nc.tensor.matmul(
                            den_ps[:, s : s + 1],
                            pt_sb[:, t, s * P : (s + 1) * P],
                            ones_sb[:],
                            start=(t == 0),
                            stop=(t == ntk - 1),
                        )
                recip_sb = misc_pool.tile([P, NBLK], F32, tag=f"recip{j}")
                nc.vector.reciprocal(recip_sb[:], den_ps[:])

                # out[tq, h] = sum_tk P^T.T @ v.  v tiles of this h-block
                # stay resident so the four s-chains use ONE psum each and
                # every normalize overlaps the next chain on PE.
                for hb in range(NBLK):
                    v_tiles = []
                    for t in range(ntk):
                        v_sb = v_pool.tile(
                            [P, BLK], BF16, tag="vs", name=f"v_{j}_{hb}_{t}"
                        )
                        nc.sync.dma_start(
                            v_sb[:],
                            v_dram[t * P : (t + 1) * P, hb * BLK : (hb + 1) * BLK],
                        )
                        v_tiles.append(v_sb)
                    for s in range(NBLK):
                        o_ps = ps_o.tile(
                            [P, BLK], F32, tag="ps_o", name=f"o_ps_{j}_{hb}_{s}"
                        )
                        for t in range(ntk):
                            nc.tensor.matmul(
                                o_ps[:],
                                pt_sb[:, t, s * P : (s + 1) * P],
                                v_tiles[t][:],
                                start=(t == 0),
                                stop=(t == ntk - 1),
                            )
                        o_sb = out_pool.tile([P, BLK], F32, tag="o")
                        nc.vector.tensor_scalar_mul(
                            o_sb[:], o_ps[:], recip_sb[:, s : s + 1]
                        )
                        nc.sync.dma_start(
                            out[
                                j * BLK + s * P : j * BLK + (s + 1) * P,
                                hb * BLK : (hb + 1) * BLK,
                            ],
                            o_sb[:],
                        )
    return nc


_DMA_TYPES = ("InstDMACopy", "InstTensorLoad", "InstTensorSave", "InstCollective")


def _split_waits(nc, limit=1):
    """This walrus build accepts only one sync-wait per TPB instruction.
    Move excess waits onto same-engine nops inserted just before the
    instruction (engine sequencers execute in order, so this is
    semantically identical)."""
    k = 0
    for f in nc.m.functions:
        for blk in f.blocks:
            new = []
            for inst in blk.instructions:
                si = inst.sync_info
                waits = list(si.on_wait) if si and si.on_wait else []
                if len(waits) > limit:
                    for w in waits[:-limit]:
                        nop = mybir.InstNoOp(name=f"wsplit-{k}", ins=[], outs=[])
                        k += 1
                        nop.engine = inst.engine
                        nop.sync_info = mybir.SyncInfo(on_wait=[w], on_update=[])
                        new.append(nop)
                    si.on_wait = waits[-limit:]
                new.append(inst)
            blk.instructions[:] = new
    return nc


_NC_CACHE = None


def _get_nc():
    global _NC_CACHE
    if _NC_CACHE is None:
        _NC_CACHE = _split_waits(_build())
    return _NC_CACHE


def _host_masks():
    # wide[p, c] = (p <= c - 511); slice [BLK-128m : 2*BLK-128m] yields the
    # partial-tile mask for diagonal offset m (p <= f - 128m + 1).
    p = np.arange(P)[:, None]
    c = np.arange(2 * BLK)[None, :]
    return (p <= c - (BLK - 1)).astype(ml_dtypes.bfloat16)


def kernel(key, query, value, Wk, Wq, Wv):
    bf = ml_dtypes.bfloat16
    wq_t = np.ascontiguousarray(Wq.T).astype(bf)  # [E, H]
    wk_t = np.ascontiguousarray(Wk.T).astype(bf)
    wv_t = np.ascontiguousarray(Wv.T).astype(bf)
    masks = _host_masks()

    in_maps = []
    for b in range(B):
        in_maps.append(
            {
                "xq": np.ascontiguousarray(query[b].T).astype(bf),
                "xk": np.ascontiguousarray(key[b].T).astype(bf),
                "xv": np.ascontiguousarray(value[b].T).astype(bf),
                "wq": wq_t,
                "wk": wk_t,
                "wv": wv_t,
                "masks": masks,
            }
        )

    nc = _get_nc()
    res = bass_utils.run_bass_kernel_spmd(nc, in_maps, core_ids=list(range(B)))
    return np.stack([res.results[i]["out"] for i in range(B)]).astype(np.float32)



# revision 2
# speedup vs baseline: 2.0424x; 2.0424x over previous
"""Trainium2 Bass kernel for a single attention head with input projections.

Per-batch-element (B=8 -> one NeuronCore each), algebraically reassociated:
  Since the head only uses q through S = q k^T, fold the two projections:
     M  = Wq^T Wk                  [E, E]   (host, shared across batch)
     u  = x_q M                    [T, E]   (instead of q = x_q Wq^T [T, H])
     S  = u x_k^T / sqrt(E)        [T, T]   (contract E=1024, not H=2048)
     P  = masked exp(S)            (kidx <= qidx + 1, one super-diagonal)
     w  = P x_v                    [T, E]   (contract T before Wv)
     out= (w Wv^T) / den           [T, H]
  This cuts per-core matmul work from ~49 GFLOP to ~24.5 GFLOP.

  The u and S GEMMs additionally run as fp8e4 (e4m3) DoubleRow matmuls
  (0.5 PE-cycles per output column per 256-deep contraction = 4x bf16
  throughput) using an error-compensated split: every operand x is held as
  x_hi = fp8(x), x_lo = fp8(x - x_hi), and x@y is computed as
  xh@yh + xl@yh + xh@yl (3 fp8 matmuls = 0.75x the bf16 cost, ~0.3% error).
  M is pre-scaled by 16 on the host so its entries sit in e4m3's normal
  range; the 1/16 is folded into the exp() input scale.  P^T, w and the
  final GEMM stay bf16 (exp spans too much dynamic range for fp8).

Layout: scores are computed TRANSPOSED (S^T tiles, tk on partitions) so
P^T feeds the w matmul directly and the softmax denominator is a
ones-matmul; w is produced as w^T [E, T] which is exactly the stationary
operand the final GEMM needs.  No on-chip transposes anywhere.
"""

import math
import sys

sys.path.insert(0, "/opt/trn_rl_repo")

import ml_dtypes
import numpy as np

import concourse.bass as bass
import concourse.mybir as mybir
import concourse.tile as tile
from concourse import bass_utils
from concourse.tile import ScopedClock

B, T, E, H = 8, 2048, 1024, 2048
P = 128
EO = E // P          # 8 e-subtiles
TKT = T // P         # 16 tk tiles
NBLK = 4             # tq blocks of 512
BLK = T // NBLK      # 512
F8 = mybir.dt.float8e4
BF16 = mybir.dt.bfloat16
F32 = mybir.dt.float32
DR = mybir.MatmulPerfMode.DoubleRow
MSCALE = 16.0                              # host scale on M (fp8 range)
EXP_SCALE = 1.0 / (MSCALE * math.sqrt(E))  # applied to S psum
EXP_BIAS = -7.0 * math.log(2.0)            # pt = exp(S/sqrt(E)) / 128


class _SplitDrainTC(tile.TileContext):
    """This walrus build rejects >1 sync-wait on the kernel-tail SP Drain
    ("Too many sync wait commands").  Spread the waits over preceding nops
    on the same engine instead — sequentially equivalent."""

    def _drain_and_barrier(self, tick_clock, wait_clock):
        nc = self.nc
        nops = [nc.sync.nop(nofuse=True) for _ in range(40)]
        drain_inst = nc.sync.drain()
        wait_clock.add_sem_waits(
            drain_inst.ins, ScopedClock({None: tick_clock.global_clock})
        )
        si = drain_inst.ins.sync_info
        waits = list(si.on_wait or [])
        if len(waits) > 1:
            assert len(waits) <= len(nops) + 1
            si.on_wait = [waits[-1]]
            for w, nop in zip(waits[:-1], nops):
                nsi = nop.ins.sync_info
                if nsi is None:
                    nop.ins.sync_info = mybir.SyncInfo(on_wait=[w], on_update=[])
                else:
                    nsi.on_wait = [w]
        nc.all_engine_barrier()
        popped = nc._tile_sem_poison_stack.pop()
        assert popped is self._sem_poison
        nc.clear_and_free_semaphores(list(self.sems.allocated().values()))
        nc.all_engine_barrier()


def _build():
    nc = bass.Bass("TRN2", target_bir_lowering=False, debug=False)

    xqh = nc.dram_tensor("xqh", (E, T), F8, kind="ExternalInput").ap()
    xql = nc.dram_tensor("xql", (E, T), F8, kind="ExternalInput").ap()
    xkh = nc.dram_tensor("xkh", (E, T), F8, kind="ExternalInput").ap()
    xkl = nc.dram_tensor("xkl", (E, T), F8, kind="ExternalInput").ap()
    mh = nc.dram_tensor("mh", (E, E), F8, kind="ExternalInput").ap()
    ml = nc.dram_tensor("ml", (E, E), F8, kind="ExternalInput").ap()
    xv = nc.dram_tensor("xv", (T, E), BF16, kind="ExternalInput").ap()
    wv = nc.dram_tensor("wv", (E, H), BF16, kind="ExternalInput").ap()
    masks = nc.dram_tensor("masks", (P, 2 * BLK), BF16, kind="ExternalInput").ap()
    out = nc.dram_tensor("out", (T, H), F32, kind="ExternalOutput").ap()

    def ko(a):  # [K, X] dram -> [128, K/128, X] view
        return a.rearrange("(ko p) t -> p ko t", p=P)

    with _SplitDrainTC(nc) as tc:
        with (
            tc.tile_pool(name="wts", bufs=1) as wts_pool,
            tc.tile_pool(name="xblk", bufs=2) as xq_pool,
            tc.tile_pool(name="useg", bufs=1) as u_pool,
            tc.tile_pool(name="pt", bufs=1) as pt_pool,
            tc.tile_pool(name="wseg", bufs=2) as w_pool,
            tc.tile_pool(name="outs", bufs=3) as out_pool,
            tc.tile_pool(name="misc", bufs=1) as misc_pool,
            tc.tile_pool(name="ps_a", bufs=4, space="PSUM") as ps_a,
            tc.tile_pool(name="ps_o", bufs=3, space="PSUM") as ps_o,
            tc.tile_pool(name="ps_d", bufs=1, space="PSUM") as ps_d,
        ):
            # ---- resident weights / activations (DMA in use-order)
            mh_sb = wts_pool.tile([P, EO, E], F8, tag="mh")
            ml_sb = wts_pool.tile([P, EO, E], F8, tag="ml")
            nc.sync.dma_start(mh_sb[:], ko(mh))
            nc.sync.dma_start(ml_sb[:], ko(ml))
            masks_sb = misc_pool.tile([P, 2 * BLK], BF16, tag="masks")
            nc.sync.dma_start(masks_sb[:], masks)
            ones_sb = misc_pool.tile([P, 1], BF16, tag="ones")
            nc.vector.memset(ones_sb[:], 1.0)
            bias_sb = misc_pool.tile([P, 1], F32, tag="bias")
            nc.vector.memset(bias_sb[:], EXP_BIAS)

            xkh_sb = wts_pool.tile([P, EO, T], F8, tag="xkh")
            xkl_sb = wts_pool.tile([P, EO, T], F8, tag="xkl")
            nc.sync.dma_start(xkh_sb[:], ko(xkh))
            nc.sync.dma_start(xkl_sb[:], ko(xkl))
            xv_sb = wts_pool.tile([P, TKT, E], BF16, tag="xv")
            for c in range(2):
                nc.sync.dma_start(
                    xv_sb[:, c * 8 : (c + 1) * 8, :],
                    ko(xv)[:, c * 8 : (c + 1) * 8, :],
                )
            wv_sb = wts_pool.tile([P, EO, H], BF16, tag="wv")
            for c in range(2):
                nc.sync.dma_start(
                    wv_sb[:, c * 4 : (c + 1) * 4, :],
                    ko(wv)[:, c * 4 : (c + 1) * 4, :],
                )

            for j in range(NBLK):
                ntk = min(4 * j + 5, TKT)  # tk tiles (mask kidx <= qidx+1)

                xqh_sb = xq_pool.tile([P, EO, BLK], F8, tag="xqh")
                xql_sb = xq_pool.tile([P, EO, BLK], F8, tag="xql")
                nc.sync.dma_start(xqh_sb[:], ko(xqh)[:, :, j * BLK : (j + 1) * BLK])
                nc.sync.dma_start(xql_sb[:], ko(xql)[:, :, j * BLK : (j + 1) * BLK])

                # ---- u^T block [128, EO, 512] as fp8 hi/lo
                uh_sb = u_pool.tile([P, EO, BLK], F8, tag="uh")
                ul_sb = u_pool.tile([P, EO, BLK], F8, tag="ul")
                for eo in range(EO):
                    ps = ps_a.tile([P, BLK], F32, tag="ps_a")
                    n = 0
                    for a_, b_ in ((mh_sb, xqh_sb), (ml_sb, xqh_sb), (mh_sb, xql_sb)):
                        for c in range(EO // 2):
                            nc.tensor.matmul(
                                ps[:],
                                a_[:, 2 * c : 2 * c + 2, eo * P : (eo + 1) * P],
                                b_[:, 2 * c : 2 * c + 2, :],
                                start=(n == 0),
                                stop=(n == 3 * EO // 2 - 1),
                                perf_mode=DR,
                            )
                            n += 1
                    nc.scalar.copy(uh_sb[:, eo, :], ps[:])
                    nc.vector.tensor_tensor(
                        ul_sb[:, eo, :], ps[:], uh_sb[:, eo, :],
                        mybir.AluOpType.subtract,
                    )

                # ---- S^T tiles -> exp -> mask -> P^T [128, ntk, 512] bf16
                pt_sb = pt_pool.tile([P, TKT, BLK], BF16)
                for t in range(ntk):
                    ps = ps_a.tile([P, BLK], F32, tag="ps_a")
                    n = 0
                    for a_, b_ in ((xkh_sb, uh_sb), (xkl_sb, uh_sb), (xkh_sb, ul_sb)):
                        for c in range(EO // 2):
                            nc.tensor.matmul(
                                ps[:],
                                a_[:, 2 * c : 2 * c + 2, t * P : (t + 1) * P],
                                b_[:, 2 * c : 2 * c + 2, :],
                                start=(n == 0),
                                stop=(n == 3 * EO // 2 - 1),
                                perf_mode=DR,
                            )
                            n += 1
                    nc.scalar.activation(
                        pt_sb[:, t, :],
                        ps[:],
                        mybir.ActivationFunctionType.Exp,
                        scale=EXP_SCALE,
                        bias=bias_sb[:],
                    )
                    m = t - 4 * j
                    if m >= 0:  # partial tile: zero the disallowed region
                        nc.vector.tensor_tensor(
                            pt_sb[:, t, :],
                            pt_sb[:, t, :],
                            masks_sb[:, BLK - m * P : 2 * BLK - m * P],
                            mybir.AluOpType.mult,
                        )

                # ---- denominator: den[tq] = sum_tk P^T ; ones-matmul
                den_ps = ps_d.tile([P, NBLK], F32)
                for s in range(NBLK):
                    for t in range(ntk):
                        nc.tensor.matmul(
                            den_ps[:, s : s + 1],
                            pt_sb[:, t, s * P : (s + 1) * P],
                            ones_sb[:],
                            start=(t == 0),
                            stop=(t == ntk - 1),
                        )
                recip_sb = misc_pool.tile([P, NBLK], F32, tag=f"recip{j}")
                nc.vector.reciprocal(recip_sb[:], den_ps[:])

                # ---- w^T block [128, EO, 512] bf16: w^T = x_v^T P^T
                w_sb = w_pool.tile([P, EO, BLK], BF16, tag="w")
                for eo in range(EO):
                    ps = ps_o.tile([P, BLK], F32, tag="ps_o")
                    for t in range(ntk):
                        nc.tensor.matmul(
                            ps[:],
                            xv_sb[:, t, eo * P : (eo + 1) * P],
                            pt_sb[:, t, :],
                            start=(t == 0),
                            stop=(t == ntk - 1),
                        )
                    nc.vector.tensor_copy(w_sb[:, eo, :], ps[:])

                # ---- out[tq, h] = (w Wv^T) * recip
                for hb in range(NBLK):
                    for s in range(NBLK):
                        o_ps = ps_o.tile(
                            [P, BLK], F32, tag="ps_o", name=f"o_ps_{j}_{hb}_{s}"
                        )
                        for eo in range(EO):
                            nc.tensor.matmul(
                                o_ps[:],
                                w_sb[:, eo, s * P : (s + 1) * P],
                                wv_sb[:, eo, hb * BLK : (hb + 1) * BLK],
                                start=(eo == 0),
                                stop=(eo == EO - 1),
                            )
                        o_sb = out_pool.tile([P, BLK], F32, tag="o")
                        nc.vector.tensor_scalar_mul(
                            o_sb[:], o_ps[:], recip_sb[:, s : s + 1]
                        )
                        nc.sync.dma_start(
                            out[
                                j * BLK + s * P : j * BLK + (s + 1) * P,
                                hb * BLK : (hb + 1) * BLK,
                            ],
                            o_sb[:],
                        )
    return nc


def _split_waits(nc, limit=1):
    """This walrus build accepts only one sync-wait per TPB instruction.
    Move excess waits onto same-engine nops inserted just before the
    instruction (engine sequencers execute in order, so this is
    semantically identical)."""
    k = 0
    for f in nc.m.functions:
        for blk in f.blocks:
            new = []
            for inst in blk.instructions:
                si = inst.sync_info
                waits = list(si.on_wait) if si and si.on_wait else []
                if len(waits) > limit:
                    for w in waits[:-limit]:
                        nop = mybir.InstNoOp(name=f"wsplit-{k}", ins=[], outs=[])
                        k += 1
                        nop.engine = inst.engine
                        nop.sync_info = mybir.SyncInfo(on_wait=[w], on_update=[])
                        new.append(nop)
                    si.on_wait = waits[-limit:]
                new.append(inst)
            blk.instructions[:] = new
    return nc


_NC_CACHE = None


def _get_nc():
    global _NC_CACHE
    if _NC_CACHE is None:
        _NC_CACHE = _split_waits(_build())
    return _NC_CACHE


def _host_masks():
    # wide[p, c] = (p <= c - 511); slice [BLK-128m : 2*BLK-128m] yields the
    # partial-tile mask for diagonal offset m (p <= f - 128m + 1).
    p = np.arange(P)[:, None]
    c = np.arange(2 * BLK)[None, :]
    return (p <= c - (BLK - 1)).astype(ml_dtypes.bfloat16)


def _split8(x):
    f8 = ml_dtypes.float8_e4m3
    hi = np.ascontiguousarray(x).astype(f8)
    lo = (x - hi.astype(np.float32)).astype(f8)
    return hi, lo


def _prep_in_maps(key, query, value, Wk, Wq, Wv):
    bf = ml_dtypes.bfloat16
    M = (MSCALE * (Wq.astype(np.float32).T @ Wk.astype(np.float32))).astype(
        np.float32
    )  # [E, E]
    mh, ml = _split8(M)
    wv_t = np.ascontiguousarray(Wv.T).astype(bf)  # [E, H]
    masks = _host_masks()

    in_maps = []
    for b in range(B):
        xqh, xql = _split8(query[b].T)
        xkh, xkl = _split8(key[b].T)
        in_maps.append(
            {
                "xqh": xqh, "xql": xql,
                "xkh": xkh, "xkl": xkl,
                "mh": mh, "ml": ml,
                "xv": value[b].astype(bf),
                "wv": wv_t,
                "masks": masks,
            }
        )
    return in_maps


def kernel(key, query, value, Wk, Wq, Wv):
    in_maps = _prep_in_maps(key, query, value, Wk, Wq, Wv)
    nc = _get_nc()
    res = bass_utils.run_bass_kernel_spmd(nc, in_maps, core_ids=list(range(B)))
    return np.stack([res.results[i]["out"] for i in range(B)]).astype(np.float32)


# revision 13
# speedup vs baseline: 2.3316x; 1.1416x over previous
"""Trainium2 Bass kernel for a single attention head with input projections.

Per-batch-element (B=8 -> one NeuronCore each), algebraically reassociated:
  Since the head only uses q through S = q k^T, fold the two projections:
     M  = Wq^T Wk                  [E, E]   (host, shared across batch)
     u  = x_q M                    [T, E]   (instead of q = x_q Wq^T [T, H])
     S  = u x_k^T / sqrt(E)        [T, T]   (contract E=1024, not H=2048)
     P  = masked exp(S)            (kidx <= qidx + 1, one super-diagonal)
     w  = P x_v                    [T, E]   (contract T before Wv)
     out= (w Wv^T) / den           [T, H]
  This cuts per-core matmul work from ~49 GFLOP to ~24.5 GFLOP.

  The u and S GEMMs additionally run as fp8e4 (e4m3) DoubleRow matmuls
  (0.5 PE-cycles per output column per 256-deep contraction = 4x bf16
  throughput) using an error-compensated split: every operand x is held as
  x_hi = fp8(x), x_lo = fp8(x - x_hi), and x@y is computed as
  xh@yh + xl@yh + xh@yl (3 fp8 matmuls = 0.75x the bf16 cost, ~0.3% error).
  M is pre-scaled by 16 on the host so its entries sit in e4m3's normal
  range; the 1/16 is folded into the exp() input scale.  P^T, w and the
  final GEMM stay bf16 (exp spans too much dynamic range for fp8).

Layout: scores are computed TRANSPOSED (S^T tiles, tk on partitions) so
P^T feeds the w matmul directly and the softmax denominator is a
ones-matmul; w is produced as w^T [E, T] which is exactly the stationary
operand the final GEMM needs.  No on-chip transposes anywhere.
"""

import math
import sys

sys.path.insert(0, "/opt/trn_rl_repo")

import ml_dtypes
import numpy as np

import concourse.bass as bass
import concourse.mybir as mybir
import concourse.tile as tile
from concourse import bass_utils
from concourse.tile import ScopedClock

B, T, E, H = 8, 2048, 1024, 2048
P = 128
EO = E // P          # 8 e-subtiles
TKT = T // P         # 16 tk tiles
NBLK = 4             # tq blocks of 512
BLK = T // NBLK      # 512
F8 = mybir.dt.float8e4
BF16 = mybir.dt.bfloat16
F32 = mybir.dt.float32
DR = mybir.MatmulPerfMode.DoubleRow
MSCALE = 16.0                              # host scale on M (fp8 range)
EXP_SCALE = 1.0 / (MSCALE * math.sqrt(E))  # applied to S psum
EXP_BIAS = -7.0 * math.log(2.0)            # pt = exp(S/sqrt(E)) / 128


class _SplitDrainTC(tile.TileContext):
    """This walrus build rejects >1 sync-wait on the kernel-tail SP Drain
    ("Too many sync wait commands").  Spread the waits over preceding nops
    on the same engine instead — sequentially equivalent."""

    def _drain_and_barrier(self, tick_clock, wait_clock):
        nc = self.nc
        nops = [nc.sync.nop(nofuse=True) for _ in range(40)]
        drain_inst = nc.sync.drain()
        wait_clock.add_sem_waits(
            drain_inst.ins, ScopedClock({None: tick_clock.global_clock})
        )
        si = drain_inst.ins.sync_info
        waits = list(si.on_wait or [])
        if len(waits) > 1:
            assert len(waits) <= len(nops) + 1
            si.on_wait = [waits[-1]]
            for w, nop in zip(waits[:-1], nops):
                nsi = nop.ins.sync_info
                if nsi is None:
                    nop.ins.sync_info = mybir.SyncInfo(on_wait=[w], on_update=[])
                else:
                    nsi.on_wait = [w]
        nc.all_engine_barrier()
        popped = nc._tile_sem_poison_stack.pop()
        assert popped is self._sem_poison
        nc.clear_and_free_semaphores(list(self.sems.allocated().values()))
        nc.all_engine_barrier()


def _build():
    nc = bass.Bass("TRN2", target_bir_lowering=False, debug=False)

    xqh = nc.dram_tensor("xqh", (E, T), F8, kind="ExternalInput").ap()
    xql = nc.dram_tensor("xql", (E, T), F8, kind="ExternalInput").ap()
    xkh = nc.dram_tensor("xkh", (E, T), F8, kind="ExternalInput").ap()
    xkl = nc.dram_tensor("xkl", (E, T), F8, kind="ExternalInput").ap()
    mh = nc.dram_tensor("mh", (E, E), F8, kind="ExternalInput").ap()
    ml = nc.dram_tensor("ml", (E, E), F8, kind="ExternalInput").ap()
    xv = nc.dram_tensor("xv", (T, E), BF16, kind="ExternalInput").ap()
    wvh = nc.dram_tensor("wvh", (E, H), F8, kind="ExternalInput").ap()
    wvl = nc.dram_tensor("wvl", (E, H), F8, kind="ExternalInput").ap()
    masks = nc.dram_tensor("masks", (P, 2 * BLK), BF16, kind="ExternalInput").ap()
    out = nc.dram_tensor("out", (T, H), F32, kind="ExternalOutput").ap()

    def ko(a):  # [K, X] dram -> [128, K/128, X] view
        return a.rearrange("(ko p) t -> p ko t", p=P)

    with _SplitDrainTC(nc) as tc:
        with (
            tc.tile_pool(name="wts", bufs=1) as wts_pool,
            tc.tile_pool(name="xblk", bufs=2) as xq_pool,
            tc.tile_pool(name="useg", bufs=1) as u_pool,
            tc.tile_pool(name="pt", bufs=1) as pt_pool,
            tc.tile_pool(name="wseg", bufs=1) as w_pool,
            tc.tile_pool(name="outs", bufs=3) as out_pool,
            tc.tile_pool(name="misc", bufs=1) as misc_pool,
            tc.tile_pool(name="ps_a", bufs=4, space="PSUM") as ps_a,
            tc.tile_pool(name="ps_o", bufs=3, space="PSUM") as ps_o,
            tc.tile_pool(name="ps_d", bufs=1, space="PSUM") as ps_d,
        ):
            # ---- resident weights / activations (DMA strictly in use-order:
            # M + xq block 0 first so PE starts ~7us in, then xk (S stage),
            # xv (w stage), wv by h-block (out stage)).
            mh_sb = wts_pool.tile([P, EO, E], F8, tag="mh")
            ml_sb = wts_pool.tile([P, EO, E], F8, tag="ml")
            xq0h_sb = xq_pool.tile([P, EO, BLK], F8, tag="xqh")
            xq0l_sb = xq_pool.tile([P, EO, BLK], F8, tag="xql")
            nc.sync.dma_start(xq0h_sb[:], ko(xqh)[:, :, 0:BLK])
            nc.sync.dma_start(mh_sb[:, :, 0 : E // 2], ko(mh)[:, :, 0 : E // 2])
            nc.sync.dma_start(mh_sb[:, :, E // 2 : E], ko(mh)[:, :, E // 2 : E])
            nc.sync.dma_start(xq0l_sb[:], ko(xql)[:, :, 0:BLK])
            nc.sync.dma_start(ml_sb[:, :, 0 : E // 2], ko(ml)[:, :, 0 : E // 2])
            nc.sync.dma_start(ml_sb[:, :, E // 2 : E], ko(ml)[:, :, E // 2 : E])
            masks_sb = misc_pool.tile([P, 2 * BLK], BF16, tag="masks")
            nc.sync.dma_start(masks_sb[:], masks)
            ones_sb = misc_pool.tile([P, 1], BF16, tag="ones")
            nc.vector.memset(ones_sb[:], 1.0)
            bias_sb = misc_pool.tile([P, 1], F32, tag="bias")
            nc.vector.memset(bias_sb[:], EXP_BIAS)

            xkh_sb = wts_pool.tile([P, EO, T], F8, tag="xkh")
            xkl_sb = wts_pool.tile([P, EO, T], F8, tag="xkl")
            nc.sync.dma_start(xkh_sb[:], ko(xkh))
            nc.sync.dma_start(xkl_sb[:], ko(xkl))
            xv_sb = wts_pool.tile([P, TKT, E], BF16, tag="xv")
            for c in range(2):
                nc.sync.dma_start(
                    xv_sb[:, c * 8 : (c + 1) * 8, :],
                    ko(xv)[:, c * 8 : (c + 1) * 8, :],
                )
            wvh_sb = wts_pool.tile([P, EO, H], F8, tag="wvh")
            wvl_sb = wts_pool.tile([P, EO, H], F8, tag="wvl")
            for c in range(NBLK):
                nc.sync.dma_start(
                    wvh_sb[:, :, c * BLK : (c + 1) * BLK],
                    ko(wvh)[:, :, c * BLK : (c + 1) * BLK],
                )
                nc.sync.dma_start(
                    wvl_sb[:, :, c * BLK : (c + 1) * BLK],
                    ko(wvl)[:, :, c * BLK : (c + 1) * BLK],
                )
            onesc_sb = misc_pool.tile([1, P], F32, tag="onesc")
            nc.vector.memset(onesc_sb[:], 1.0)

            for j in range(NBLK):
                ntk = min(4 * j + 5, TKT)  # tk tiles (mask kidx <= qidx+1)

                if j == 0:
                    xqh_sb, xql_sb = xq0h_sb, xq0l_sb
                else:
                    xqh_sb = xq_pool.tile([P, EO, BLK], F8, tag="xqh")
                    xql_sb = xq_pool.tile([P, EO, BLK], F8, tag="xql")
                    nc.sync.dma_start(
                        xqh_sb[:], ko(xqh)[:, :, j * BLK : (j + 1) * BLK]
                    )
                    nc.sync.dma_start(
                        xql_sb[:], ko(xql)[:, :, j * BLK : (j + 1) * BLK]
                    )

                # ---- u^T block [128, EO, 512] as fp8 hi/lo
                uh_sb = u_pool.tile([P, EO, BLK], F8, tag="uh")
                ul_sb = u_pool.tile([P, EO, BLK], F8, tag="ul")
                for eo in range(EO):
                    ps = ps_a.tile([P, BLK], F32, tag="ps_a")
                    n = 0
                    for a_, b_ in ((mh_sb, xqh_sb), (mh_sb, xql_sb), (ml_sb, xqh_sb)):
                        for c in range(EO // 2):
                            nc.tensor.matmul(
                                ps[:],
                                a_[:, 2 * c : 2 * c + 2, eo * P : (eo + 1) * P],
                                b_[:, 2 * c : 2 * c + 2, :],
                                start=(n == 0),
                                stop=(n == 3 * EO // 2 - 1),
                                perf_mode=DR,
                            )
                            n += 1
                    nc.scalar.copy(uh_sb[:, eo, :], ps[:])
                    nc.vector.tensor_tensor(
                        ul_sb[:, eo, :], ps[:], uh_sb[:, eo, :],
                        mybir.AluOpType.subtract,
                    )

                # ---- S^T tiles -> exp -> mask -> P^T [128, ntk, 512] bf16
                # den-row accumulates per tile: den[1, tq] += ones^T P^T
                pt_sb = pt_pool.tile([P, TKT, BLK], BF16)
                den_ps = ps_d.tile([1, BLK], F32)
                for t in range(ntk):
                    ps = ps_a.tile([P, BLK], F32, tag="ps_a")
                    # last e-pair last: covers the Act/DVE split tail of the
                    # final u chain (eo=6,7) with 9 instructions of work
                    n = 0
                    s_terms = ((xkh_sb, uh_sb), (xkl_sb, uh_sb), (xkh_sb, ul_sb))
                    for c in (0, 1, 2, 3):
                        for a_, b_ in s_terms if c < 3 else s_terms:
                            pass
                    for c in (0, 1, 2):
                        for a_, b_ in s_terms:
                            nc.tensor.matmul(
                                ps[:],
                                a_[:, 2 * c : 2 * c + 2, t * P : (t + 1) * P],
                                b_[:, 2 * c : 2 * c + 2, :],
                                start=(n == 0),
                                stop=False,
                                perf_mode=DR,
                            )
                            n += 1
                    for a_, b_ in s_terms:
                        nc.tensor.matmul(
                            ps[:],
                            a_[:, 6:8, t * P : (t + 1) * P],
                            b_[:, 6:8, :],
                            start=False,
                            stop=(n == 3 * EO // 2 - 1),
                            perf_mode=DR,
                        )
                        n += 1
                    nc.scalar.activation(
                        pt_sb[:, t, :],
                        ps[:],
                        mybir.ActivationFunctionType.Exp,
                        scale=EXP_SCALE,
                        bias=bias_sb[:],
                    )
                    m = t - 4 * j
                    if m >= 0:  # partial tile: zero the disallowed region
                        nc.vector.tensor_tensor(
                            pt_sb[:, t, :],
                            pt_sb[:, t, :],
                            masks_sb[:, BLK - m * P : 2 * BLK - m * P],
                            mybir.AluOpType.mult,
                        )
                    nc.tensor.matmul(
                        den_ps[:],
                        ones_sb[:],
                        pt_sb[:, t, :],
                        start=(t == 0),
                        stop=(t == ntk - 1),
                    )

                # sigma[e, tq] = 1/den[tq] broadcast along partitions (PE
                # outer product with a ones column), for normalizing w^T.
                recip_sb = misc_pool.tile([1, BLK], F32, tag="recip")
                nc.vector.reciprocal(recip_sb[:], den_ps[:])
                sg_ps = ps_o.tile([P, BLK], F32, tag="ps_o", name=f"sg_ps_{j}")
                nc.tensor.matmul(sg_ps[:], onesc_sb[:], recip_sb[:])
                sg_sb = misc_pool.tile([P, BLK], F32, tag="sg")
                nc.vector.tensor_copy(sg_sb[:], sg_ps[:])

                # ---- w^T block: w^T = x_v^T P^T, normalized by sigma and
                # split to fp8 hi/lo for the DoubleRow out-stage.
                w_sb = w_pool.tile([P, EO, BLK], BF16, tag="w")
                wh_sb = w_pool.tile([P, EO, BLK], F8, tag="wh")
                wl_sb = w_pool.tile([P, EO, BLK], F8, tag="wl")
                for eo in range(EO):
                    ps = ps_o.tile([P, BLK], F32, tag="ps_o")
                    for t in range(ntk):
                        nc.tensor.matmul(
                            ps[:],
                            xv_sb[:, t, eo * P : (eo + 1) * P],
                            pt_sb[:, t, :],
                            start=(t == 0),
                            stop=(t == ntk - 1),
                        )
                    nc.vector.tensor_tensor(
                        w_sb[:, eo, :], ps[:], sg_sb[:], mybir.AluOpType.mult
                    )
                    nc.scalar.copy(wh_sb[:, eo, :], w_sb[:, eo, :])
                    nc.vector.tensor_tensor(
                        wl_sb[:, eo, :], w_sb[:, eo, :], wh_sb[:, eo, :],
                        mybir.AluOpType.subtract,
                    )

                # ---- out[tq, h] = (w_n (16 Wv)^T) / 16
                for hb in range(NBLK):
                    for s in range(NBLK):
                        o_ps = ps_o.tile(
                            [P, BLK], F32, tag="ps_o", name=f"o_ps_{j}_{hb}_{s}"
                        )
                        n = 0
                        for a_, b_ in (
                            (wh_sb, wvh_sb), (wh_sb, wvl_sb), (wl_sb, wvh_sb)
                        ):
                            for c in range(EO // 2):
                                nc.tensor.matmul(
                                    o_ps[:],
                                    a_[:, 2 * c : 2 * c + 2, s * P : (s + 1) * P],
                                    b_[:, 2 * c : 2 * c + 2, hb * BLK : (hb + 1) * BLK],
                                    start=(n == 0),
                                    stop=(n == 3 * EO // 2 - 1),
                                    perf_mode=DR,
                                )
                                n += 1
                        o_sb = out_pool.tile([P, BLK], F32, tag="o")
                        nc.vector.tensor_scalar_mul(o_sb[:], o_ps[:], 1.0 / 16.0)
                        nc.sync.dma_start(
                            out[
                                j * BLK + s * P : j * BLK + (s + 1) * P,
                                hb * BLK : (hb + 1) * BLK,
                            ],
                            o_sb[:],
                        )
    return nc


def _split_waits(nc, limit=1):
    """This walrus build accepts only one sync-wait per TPB instruction.
    Move excess waits onto same-engine nops inserted just before the
    instruction (engine sequencers execute in order, so this is
    semantically identical)."""
    k = 0
    for f in nc.m.functions:
        for blk in f.blocks:
            new = []
            for inst in blk.instructions:
                si = inst.sync_info
                waits = list(si.on_wait) if si and si.on_wait else []
                if len(waits) > limit:
                    for w in waits[:-limit]:
                        nop = mybir.InstNoOp(name=f"wsplit-{k}", ins=[], outs=[])
                        k += 1
                        nop.engine = inst.engine
                        nop.sync_info = mybir.SyncInfo(on_wait=[w], on_update=[])
                        new.append(nop)
                    si.on_wait = waits[-limit:]
                new.append(inst)
            blk.instructions[:] = new
    return nc


_NC_CACHE = None


def _get_nc():
    global _NC_CACHE
    if _NC_CACHE is None:
        _NC_CACHE = _split_waits(_build())
    return _NC_CACHE


def _host_masks():
    # wide[p, c] = (p <= c - 511); slice [BLK-128m : 2*BLK-128m] yields the
    # partial-tile mask for diagonal offset m (p <= f - 128m + 1).
    p = np.arange(P)[:, None]
    c = np.arange(2 * BLK)[None, :]
    return (p <= c - (BLK - 1)).astype(ml_dtypes.bfloat16)


def _split8(x):
    f8 = ml_dtypes.float8_e4m3
    hi = np.ascontiguousarray(x).astype(f8)
    lo = (x - hi.astype(np.float32)).astype(f8)
    return hi, lo


def _prep_in_maps(key, query, value, Wk, Wq, Wv):
    bf = ml_dtypes.bfloat16
    M = (MSCALE * (Wq.astype(np.float32).T @ Wk.astype(np.float32))).astype(
        np.float32
    )  # [E, E]
    mh, ml = _split8(M)
    wvh, wvl = _split8((MSCALE * Wv.T).astype(np.float32))  # [E, H], 16x
    masks = _host_masks()

    in_maps = []
    for b in range(B):
        xqh, xql = _split8(query[b].T)
        xkh, xkl = _split8(key[b].T)
        in_maps.append(
            {
                "xqh": xqh, "xql": xql,
                "xkh": xkh, "xkl": xkl,
                "mh": mh, "ml": ml,
                "xv": value[b].astype(bf),
                "wvh": wvh, "wvl": wvl,
                "masks": masks,
            }
        )
    return in_maps


def kernel(key, query, value, Wk, Wq, Wv):
    in_maps = _prep_in_maps(key, query, value, Wk, Wq, Wv)
    nc = _get_nc()
    res = bass_utils.run_bass_kernel_spmd(nc, in_maps, core_ids=list(range(B)))
    return np.stack([res.results[i]["out"] for i in range(B)]).astype(np.float32)


# revision 18
# speedup vs baseline: 2.3977x; 1.0283x over previous
"""Trainium2 Bass kernel for a single attention head with input projections.

Per-batch-element (B=8 -> one NeuronCore each), algebraically reassociated:
  Since the head only uses q through S = q k^T, fold the two projections:
     M  = Wq^T Wk                  [E, E]   (host, shared across batch)
     u  = x_q M                    [T, E]   (instead of q = x_q Wq^T [T, H])
     S  = u x_k^T / sqrt(E)        [T, T]   (contract E=1024, not H=2048)
     P  = masked exp(S)            (kidx <= qidx + 1, one super-diagonal)
     w  = P x_v                    [T, E]   (contract T before Wv)
     out= (w Wv^T) / den           [T, H]
  This cuts per-core matmul work from ~49 GFLOP to ~24.5 GFLOP.

  The u and S GEMMs additionally run as fp8e4 (e4m3) DoubleRow matmuls
  (0.5 PE-cycles per output column per 256-deep contraction = 4x bf16
  throughput) using an error-compensated split: every operand x is held as
  x_hi = fp8(x), x_lo = fp8(x - x_hi), and x@y is computed as
  xh@yh + xl@yh + xh@yl (3 fp8 matmuls = 0.75x the bf16 cost, ~0.3% error).
  M is pre-scaled by 16 on the host so its entries sit in e4m3's normal
  range; the 1/16 is folded into the exp() input scale.  P^T, w and the
  final GEMM stay bf16 (exp spans too much dynamic range for fp8).

Layout: scores are computed TRANSPOSED (S^T tiles, tk on partitions) so
P^T feeds the w matmul directly and the softmax denominator is a
ones-matmul; w is produced as w^T [E, T] which is exactly the stationary
operand the final GEMM needs.  No on-chip transposes anywhere.
"""

import math
import sys

sys.path.insert(0, "/opt/trn_rl_repo")

import ml_dtypes
import numpy as np

import concourse.bass as bass
import concourse.mybir as mybir
import concourse.tile as tile
from concourse import bass_utils
from concourse.tile import ScopedClock

B, T, E, H = 8, 2048, 1024, 2048
P = 128
EO = E // P          # 8 e-subtiles
TKT = T // P         # 16 tk tiles
NBLK = 4             # tq blocks of 512
BLK = T // NBLK      # 512
F8 = mybir.dt.float8e4
BF16 = mybir.dt.bfloat16
F32 = mybir.dt.float32
DR = mybir.MatmulPerfMode.DoubleRow
MSCALE = 16.0                              # host scale on M (fp8 range)
EXP_SCALE = 1.0 / (MSCALE * math.sqrt(E))  # applied to S psum
EXP_BIAS = -7.0 * math.log(2.0)            # pt = exp(S/sqrt(E)) / 128


class _SplitDrainTC(tile.TileContext):
    """This walrus build rejects >1 sync-wait on the kernel-tail SP Drain
    ("Too many sync wait commands").  Spread the waits over preceding nops
    on the same engine instead — sequentially equivalent."""

    def _drain_and_barrier(self, tick_clock, wait_clock):
        nc = self.nc
        nops = [nc.sync.nop(nofuse=True) for _ in range(40)]
        drain_inst = nc.sync.drain()
        wait_clock.add_sem_waits(
            drain_inst.ins, ScopedClock({None: tick_clock.global_clock})
        )
        si = drain_inst.ins.sync_info
        waits = list(si.on_wait or [])
        if len(waits) > 1:
            assert len(waits) <= len(nops) + 1
            si.on_wait = [waits[-1]]
            for w, nop in zip(waits[:-1], nops):
                nsi = nop.ins.sync_info
                if nsi is None:
                    nop.ins.sync_info = mybir.SyncInfo(on_wait=[w], on_update=[])
                else:
                    nsi.on_wait = [w]
        nc.all_engine_barrier()
        popped = nc._tile_sem_poison_stack.pop()
        assert popped is self._sem_poison
        nc.clear_and_free_semaphores(list(self.sems.allocated().values()))
        nc.all_engine_barrier()


def _build():
    nc = bass.Bass("TRN2", target_bir_lowering=False, debug=False)

    xqh = nc.dram_tensor("xqh", (E, T), F8, kind="ExternalInput").ap()
    xql = nc.dram_tensor("xql", (E, T), F8, kind="ExternalInput").ap()
    xkh = nc.dram_tensor("xkh", (E, T), F8, kind="ExternalInput").ap()
    xkl = nc.dram_tensor("xkl", (E, T), F8, kind="ExternalInput").ap()
    mh = nc.dram_tensor("mh", (E, E), F8, kind="ExternalInput").ap()
    ml = nc.dram_tensor("ml", (E, E), F8, kind="ExternalInput").ap()
    xv = nc.dram_tensor("xv", (T, E), BF16, kind="ExternalInput").ap()
    wvh = nc.dram_tensor("wvh", (E, H), F8, kind="ExternalInput").ap()
    wvl = nc.dram_tensor("wvl", (E, H), F8, kind="ExternalInput").ap()
    masks = nc.dram_tensor("masks", (P, 2 * BLK), BF16, kind="ExternalInput").ap()
    out = nc.dram_tensor("out", (T, H), F32, kind="ExternalOutput").ap()

    def ko(a):  # [K, X] dram -> [128, K/128, X] view
        return a.rearrange("(ko p) t -> p ko t", p=P)

    with _SplitDrainTC(nc) as tc:
        with (
            tc.tile_pool(name="wts", bufs=1) as wts_pool,
            tc.tile_pool(name="xblk", bufs=2) as xq_pool,
            tc.tile_pool(name="useg", bufs=1) as u_pool,
            tc.tile_pool(name="pt", bufs=1) as pt_pool,
            tc.tile_pool(name="wseg", bufs=1) as w_pool,
            tc.tile_pool(name="outs", bufs=3) as out_pool,
            tc.tile_pool(name="misc", bufs=1) as misc_pool,
            tc.tile_pool(name="ps_a", bufs=4, space="PSUM") as ps_a,
            tc.tile_pool(name="ps_o", bufs=3, space="PSUM") as ps_o,
            tc.tile_pool(name="ps_d", bufs=1, space="PSUM") as ps_d,
        ):
            # ---- resident weights / activations (DMA strictly in use-order:
            # M + xq block 0 first so PE starts ~7us in, then xk (S stage),
            # xv (w stage), wv by h-block (out stage)).
            mh_sb = wts_pool.tile([P, EO, E], F8, tag="mh")
            ml_sb = wts_pool.tile([P, EO, E], F8, tag="ml")
            xq0h_sb = xq_pool.tile([P, EO, BLK], F8, tag="xqh")
            xq0l_sb = xq_pool.tile([P, EO, BLK], F8, tag="xql")
            nc.sync.dma_start(xq0h_sb[:], ko(xqh)[:, :, 0:BLK])
            nc.sync.dma_start(mh_sb[:, :, 0 : E // 2], ko(mh)[:, :, 0 : E // 2])
            nc.sync.dma_start(mh_sb[:, :, E // 2 : E], ko(mh)[:, :, E // 2 : E])
            nc.sync.dma_start(xq0l_sb[:], ko(xql)[:, :, 0:BLK])
            nc.sync.dma_start(ml_sb[:, :, 0 : E // 2], ko(ml)[:, :, 0 : E // 2])
            nc.sync.dma_start(ml_sb[:, :, E // 2 : E], ko(ml)[:, :, E // 2 : E])
            masks_sb = misc_pool.tile([P, 2 * BLK], BF16, tag="masks")
            nc.sync.dma_start(masks_sb[:], masks)
            ones_sb = misc_pool.tile([P, P], BF16, tag="ones")
            nc.vector.memset(ones_sb[:], 1.0)
            bias_sb = misc_pool.tile([P, 1], F32, tag="bias")
            nc.vector.memset(bias_sb[:], EXP_BIAS)
            bias1_sb = misc_pool.tile([1, 1], F32, tag="bias1")
            nc.vector.memset(bias1_sb[:], EXP_BIAS)
            onesr_sb = misc_pool.tile([1, P], BF16, tag="onesr")
            nc.vector.memset(onesr_sb[:], 1.0)

            xkh_sb = wts_pool.tile([P, EO, T], F8, tag="xkh")
            xkl_sb = wts_pool.tile([P, EO, T], F8, tag="xkl")
            nc.sync.dma_start(xkh_sb[:], ko(xkh))
            nc.sync.dma_start(xkl_sb[:], ko(xkl))
            xv_sb = wts_pool.tile([P, TKT, E], BF16, tag="xv")
            for c in range(2):
                nc.sync.dma_start(
                    xv_sb[:, c * 8 : (c + 1) * 8, :],
                    ko(xv)[:, c * 8 : (c + 1) * 8, :],
                )
            wvh_sb = wts_pool.tile([P, EO, H], F8, tag="wvh")
            wvl_sb = wts_pool.tile([P, EO, H], F8, tag="wvl")
            for c in range(NBLK):
                nc.sync.dma_start(
                    wvh_sb[:, :, c * BLK : (c + 1) * BLK],
                    ko(wvh)[:, :, c * BLK : (c + 1) * BLK],
                )
                nc.sync.dma_start(
                    wvl_sb[:, :, c * BLK : (c + 1) * BLK],
                    ko(wvl)[:, :, c * BLK : (c + 1) * BLK],
                )

            def emit_u_stage(j, xqh_sb, xql_sb):
                # ---- u^T block [128, EO, 512] as fp8 hi/lo.  For j > 0 this
                # is emitted between w(j-1) and out(j-1): the u chains fill
                # the w-split PE bubble and out(j-1) covers the u-split tail.
                if xqh_sb is None:
                    xqh_sb = xq_pool.tile([P, EO, BLK], F8, tag="xqh")
                    xql_sb = xq_pool.tile([P, EO, BLK], F8, tag="xql")
                    nc.sync.dma_start(
                        xqh_sb[:], ko(xqh)[:, :, j * BLK : (j + 1) * BLK]
                    )
                    nc.sync.dma_start(
                        xql_sb[:], ko(xql)[:, :, j * BLK : (j + 1) * BLK]
                    )
                uh_sb = u_pool.tile([P, EO, BLK], F8, tag="uh", name=f"uh{j}")
                ul_sb = u_pool.tile([P, EO, BLK], F8, tag="ul", name=f"ul{j}")
                for eo in range(EO):
                    ps = ps_a.tile([P, BLK], F32, tag="ps_a")
                    n = 0
                    for a_, b_ in ((mh_sb, xqh_sb), (mh_sb, xql_sb), (ml_sb, xqh_sb)):
                        for c in range(EO // 2):
                            nc.tensor.matmul(
                                ps[:],
                                a_[:, 2 * c : 2 * c + 2, eo * P : (eo + 1) * P],
                                b_[:, 2 * c : 2 * c + 2, :],
                                start=(n == 0),
                                stop=(n == 3 * EO // 2 - 1),
                                perf_mode=DR,
                            )
                            n += 1
                    nc.scalar.copy(uh_sb[:, eo, :], ps[:])
                    nc.vector.tensor_tensor(
                        ul_sb[:, eo, :], ps[:], uh_sb[:, eo, :],
                        mybir.AluOpType.subtract,
                    )
                return uh_sb, ul_sb

            u_next = emit_u_stage(0, xq0h_sb, xq0l_sb)
            for j in range(NBLK):
                # The mask kidx <= qidx+1 needs tiles tk <= 4j+4, but tile
                # 4j+4 contains exactly ONE valid element (q = 512j+511,
                # k = 512j+512).  Handle it as a [1,1] scalar side-path and
                # run the dense loops over ntk = 4j+4 tiles only.
                has_sd = 4 * j + 4 < TKT
                ntk = 4 * j + 4 if has_sd else TKT
                uh_sb, ul_sb = u_next

                # ---- S^T tiles -> exp -> mask -> P^T [128, ntk, 512] bf16
                # den-row accumulates per tile: den[1, tq] += ones^T P^T
                pt_sb = pt_pool.tile([P, TKT, BLK], BF16)
                den_ps = ps_d.tile([P, BLK], F32)
                for t in range(ntk):
                    ps = ps_a.tile([P, BLK], F32, tag="ps_a")
                    # last e-pair last: covers the Act/DVE split tail of the
                    # final u chain (eo=6,7) with 9 instructions of work
                    n = 0
                    s_terms = ((xkh_sb, uh_sb), (xkl_sb, uh_sb), (xkh_sb, ul_sb))
                    for c in (0, 1, 2):
                        for a_, b_ in s_terms:
                            nc.tensor.matmul(
                                ps[:],
                                a_[:, 2 * c : 2 * c + 2, t * P : (t + 1) * P],
                                b_[:, 2 * c : 2 * c + 2, :],
                                start=(n == 0),
                                stop=False,
                                perf_mode=DR,
                            )
                            n += 1
                    for a_, b_ in s_terms:
                        nc.tensor.matmul(
                            ps[:],
                            a_[:, 6:8, t * P : (t + 1) * P],
                            b_[:, 6:8, :],
                            start=False,
                            stop=(n == 3 * EO // 2 - 1),
                            perf_mode=DR,
                        )
                        n += 1
                    nc.scalar.activation(
                        pt_sb[:, t, :],
                        ps[:],
                        mybir.ActivationFunctionType.Exp,
                        scale=EXP_SCALE,
                        bias=bias_sb[:],
                    )
                    m = t - 4 * j
                    if m >= 0:  # partial tile: zero the disallowed region
                        nc.vector.tensor_tensor(
                            pt_sb[:, t, :],
                            pt_sb[:, t, :],
                            masks_sb[:, BLK - m * P : 2 * BLK - m * P],
                            mybir.AluOpType.mult,
                        )
                    nc.tensor.matmul(
                        den_ps[:],
                        ones_sb[:],
                        pt_sb[:, t, :],
                        start=(t == 0),
                        stop=(not has_sd and t == ntk - 1),
                    )

                if has_sd:
                    # scalar S element: S[q0, k0], q0 = 512j+511, k0 = 512j+512
                    k0 = ntk * P  # column of k0 in the [E, T] layouts
                    sd_ps = ps_a.tile([1, 1], F32, tag="sd", name=f"sd_ps_{j}")
                    n = 0
                    for a_, b_ in ((xkh_sb, uh_sb), (xkl_sb, uh_sb), (xkh_sb, ul_sb)):
                        for c in range(EO // 2):
                            nc.tensor.matmul(
                                sd_ps[:],
                                a_[:, 2 * c : 2 * c + 2, k0 : k0 + 1],
                                b_[:, 2 * c : 2 * c + 2, BLK - 1 : BLK],
                                start=(n == 0),
                                stop=(n == 3 * EO // 2 - 1),
                                perf_mode=DR,
                            )
                            n += 1
                    pt1_sb = misc_pool.tile([1, 1], BF16, tag="pt1")
                    nc.scalar.activation(
                        pt1_sb[:],
                        sd_ps[:],
                        mybir.ActivationFunctionType.Exp,
                        scale=EXP_SCALE,
                        bias=bias1_sb[:],
                    )
                    nc.tensor.matmul(
                        den_ps[:, BLK - 1 : BLK],
                        onesr_sb[:],
                        pt1_sb[:],
                        start=False,
                        stop=True,
                        skip_group_check=True,
                    )

                # sigma[e, tq] = 1/den[tq], already replicated across
                # partitions because the den matmul used a full ones matrix.
                sg_sb = misc_pool.tile([P, BLK], F32, tag="sg")
                nc.vector.reciprocal(sg_sb[:], den_ps[:])

                # ---- w^T block: w^T = x_v^T P^T, normalized by sigma and
                # split to fp8 hi/lo for the DoubleRow out-stage.
                w_sb = w_pool.tile([P, EO, BLK], BF16, tag="w")
                wh_sb = w_pool.tile([P, EO, BLK], F8, tag="wh")
                wl_sb = w_pool.tile([P, EO, BLK], F8, tag="wl")
                for eo in range(EO):
                    ps = ps_o.tile([P, BLK], F32, tag="ps_o")
                    for t in range(ntk):
                        nc.tensor.matmul(
                            ps[:],
                            xv_sb[:, t, eo * P : (eo + 1) * P],
                            pt_sb[:, t, :],
                            start=(t == 0),
                            stop=(not has_sd and t == ntk - 1),
                        )
                    if has_sd:
                        nc.tensor.matmul(
                            ps[:, BLK - 1 : BLK],
                            xv_sb[0:1, ntk, eo * P : (eo + 1) * P],
                            pt1_sb[:],
                            start=False,
                            stop=True,
                            skip_group_check=True,
                        )
                    nc.vector.tensor_tensor(
                        w_sb[:, eo, :], ps[:], sg_sb[:], mybir.AluOpType.mult
                    )
                    nc.scalar.copy(wh_sb[:, eo, :], w_sb[:, eo, :])
                    nc.vector.tensor_tensor(
                        wl_sb[:, eo, :], w_sb[:, eo, :], wh_sb[:, eo, :],
                        mybir.AluOpType.subtract,
                    )

                if j + 1 < NBLK:
                    u_next = emit_u_stage(j + 1, None, None)

                # ---- out[tq, h] = (w_n (16 Wv)^T) / 16
                for hb in range(NBLK):
                    for s in range(NBLK):
                        o_ps = ps_o.tile(
                            [P, BLK], F32, tag="ps_o", name=f"o_ps_{j}_{hb}_{s}"
                        )
                        n = 0
                        o_terms = (
                            (wh_sb, wvh_sb), (wh_sb, wvl_sb), (wl_sb, wvh_sb)
                        )
                        for c in (0, 1, 2):
                            for a_, b_ in o_terms:
                                nc.tensor.matmul(
                                    o_ps[:],
                                    a_[:, 2 * c : 2 * c + 2, s * P : (s + 1) * P],
                                    b_[:, 2 * c : 2 * c + 2, hb * BLK : (hb + 1) * BLK],
                                    start=(n == 0),
                                    stop=False,
                                    perf_mode=DR,
                                )
                                n += 1
                        for a_, b_ in o_terms:
                            nc.tensor.matmul(
                                o_ps[:],
                                a_[:, 6:8, s * P : (s + 1) * P],
                                b_[:, 6:8, hb * BLK : (hb + 1) * BLK],
                                start=False,
                                stop=(n == 3 * EO // 2 - 1),
                                perf_mode=DR,
                            )
                            n += 1
                        o_sb = out_pool.tile([P, BLK], F32, tag="o")
                        nc.vector.tensor_scalar_mul(o_sb[:], o_ps[:], 1.0 / 16.0)
                        nc.sync.dma_start(
                            out[
                                j * BLK + s * P : j * BLK + (s + 1) * P,
                                hb * BLK : (hb + 1) * BLK,
                            ],
                            o_sb[:],
                        )
    return nc


def _split_waits(nc, limit=1):
    """This walrus build accepts only one sync-wait per TPB instruction.
    Move excess waits onto same-engine nops inserted just before the
    instruction (engine sequencers execute in order, so this is
    semantically identical)."""
    k = 0
    for f in nc.m.functions:
        for blk in f.blocks:
            new = []
            for inst in blk.instructions:
                si = inst.sync_info
                waits = list(si.on_wait) if si and si.on_wait else []
                if len(waits) > limit:
                    for w in waits[:-limit]:
                        nop = mybir.InstNoOp(name=f"wsplit-{k}", ins=[], outs=[])
                        k += 1
                        nop.engine = inst.engine
                        nop.sync_info = mybir.SyncInfo(on_wait=[w], on_update=[])
                        new.append(nop)
                    si.on_wait = waits[-limit:]
                new.append(inst)
            blk.instructions[:] = new
    return nc


_NC_CACHE = None


def _get_nc():
    global _NC_CACHE
    if _NC_CACHE is None:
        _NC_CACHE = _split_waits(_build())
    return _NC_CACHE


def _host_masks():
    # wide[p, c] = (p <= c - 511); slice [BLK-128m : 2*BLK-128m] yields the
    # partial-tile mask for diagonal offset m (p <= f - 128m + 1).
    p = np.arange(P)[:, None]
    c = np.arange(2 * BLK)[None, :]
    return (p <= c - (BLK - 1)).astype(ml_dtypes.bfloat16)


def _split8(x):
    f8 = ml_dtypes.float8_e4m3
    hi = np.ascontiguousarray(x).astype(f8)
    lo = (x - hi.astype(np.float32)).astype(f8)
    return hi, lo


def _prep_in_maps(key, query, value, Wk, Wq, Wv):
    bf = ml_dtypes.bfloat16
    M = (MSCALE * (Wq.astype(np.float32).T @ Wk.astype(np.float32))).astype(
        np.float32
    )  # [E, E]
    mh, ml = _split8(M)
    wvh, wvl = _split8((MSCALE * Wv.T).astype(np.float32))  # [E, H], 16x
    masks = _host_masks()

    in_maps = []
    for b in range(B):
        xqh, xql = _split8(query[b].T)
        xkh, xkl = _split8(key[b].T)
        in_maps.append(
            {
                "xqh": xqh, "xql": xql,
                "xkh": xkh, "xkl": xkl,
                "mh": mh, "ml": ml,
                "xv": value[b].astype(bf),
                "wvh": wvh, "wvl": wvl,
                "masks": masks,
            }
        )
    return in_maps


def kernel(key, query, value, Wk, Wq, Wv):
    in_maps = _prep_in_maps(key, query, value, Wk, Wq, Wv)
    nc = _get_nc()
    res = bass_utils.run_bass_kernel_spmd(nc, in_maps, core_ids=list(range(B)))
    return np.stack([res.results[i]["out"] for i in range(B)]).astype(np.float32)


# revision 20
# speedup vs baseline: 2.4559x; 1.0243x over previous
"""Trainium2 Bass kernel for a single attention head with input projections.

Per-batch-element (B=8 -> one NeuronCore each), algebraically reassociated:
  Since the head only uses q through S = q k^T, fold the two projections:
     M  = Wq^T Wk                  [E, E]   (host, shared across batch)
     u  = x_q M                    [T, E]   (instead of q = x_q Wq^T [T, H])
     S  = u x_k^T / sqrt(E)        [T, T]   (contract E=1024, not H=2048)
     P  = masked exp(S)            (kidx <= qidx + 1, one super-diagonal)
     w  = P x_v                    [T, E]   (contract T before Wv)
     out= (w Wv^T) / den           [T, H]
  This cuts per-core matmul work from ~49 GFLOP to ~24.5 GFLOP.

  The u and S GEMMs additionally run as fp8e4 (e4m3) DoubleRow matmuls
  (0.5 PE-cycles per output column per 256-deep contraction = 4x bf16
  throughput) using an error-compensated split: every operand x is held as
  x_hi = fp8(x), x_lo = fp8(x - x_hi), and x@y is computed as
  xh@yh + xl@yh + xh@yl (3 fp8 matmuls = 0.75x the bf16 cost, ~0.3% error).
  M is pre-scaled by 16 on the host so its entries sit in e4m3's normal
  range; the 1/16 is folded into the exp() input scale.  P^T, w and the
  final GEMM stay bf16 (exp spans too much dynamic range for fp8).

Layout: scores are computed TRANSPOSED (S^T tiles, tk on partitions) so
P^T feeds the w matmul directly and the softmax denominator is a
ones-matmul; w is produced as w^T [E, T] which is exactly the stationary
operand the final GEMM needs.  No on-chip transposes anywhere.
"""

import math
import sys

sys.path.insert(0, "/opt/trn_rl_repo")

import ml_dtypes
import numpy as np

import concourse.bass as bass
import concourse.mybir as mybir
import concourse.tile as tile
from concourse import bass_utils
from concourse.tile import ScopedClock

B, T, E, H = 8, 2048, 1024, 2048
P = 128
EO = E // P          # 8 e-subtiles
TKT = T // P         # 16 tk tiles
NBLK = 4             # tq blocks of 512
BLK = T // NBLK      # 512
F8 = mybir.dt.float8e4
BF16 = mybir.dt.bfloat16
F32 = mybir.dt.float32
DR = mybir.MatmulPerfMode.DoubleRow
MSCALE = 16.0                              # host scale on M (fp8 range)
EXP_SCALE = 1.0 / (MSCALE * math.sqrt(E))  # applied to S psum
EXP_BIAS = -7.0 * math.log(2.0)            # pt = exp(S/sqrt(E)) / 128


class _SplitDrainTC(tile.TileContext):
    """This walrus build rejects >1 sync-wait on the kernel-tail SP Drain
    ("Too many sync wait commands").  Spread the waits over preceding nops
    on the same engine instead — sequentially equivalent."""

    def _drain_and_barrier(self, tick_clock, wait_clock):
        nc = self.nc
        nops = [nc.sync.nop(nofuse=True) for _ in range(40)]
        drain_inst = nc.sync.drain()
        wait_clock.add_sem_waits(
            drain_inst.ins, ScopedClock({None: tick_clock.global_clock})
        )
        si = drain_inst.ins.sync_info
        waits = list(si.on_wait or [])
        if len(waits) > 1:
            assert len(waits) <= len(nops) + 1
            si.on_wait = [waits[-1]]
            for w, nop in zip(waits[:-1], nops):
                nsi = nop.ins.sync_info
                if nsi is None:
                    nop.ins.sync_info = mybir.SyncInfo(on_wait=[w], on_update=[])
                else:
                    nsi.on_wait = [w]
        nc.all_engine_barrier()
        popped = nc._tile_sem_poison_stack.pop()
        assert popped is self._sem_poison
        nc.clear_and_free_semaphores(list(self.sems.allocated().values()))
        nc.all_engine_barrier()


def _build():
    nc = bass.Bass("TRN2", target_bir_lowering=False, debug=False)

    xqh = nc.dram_tensor("xqh", (E, T), F8, kind="ExternalInput").ap()
    xql = nc.dram_tensor("xql", (E, T), F8, kind="ExternalInput").ap()
    xkh = nc.dram_tensor("xkh", (E, T), F8, kind="ExternalInput").ap()
    xkl = nc.dram_tensor("xkl", (E, T), F8, kind="ExternalInput").ap()
    mh = nc.dram_tensor("mh", (E, E), F8, kind="ExternalInput").ap()
    ml = nc.dram_tensor("ml", (E, E), F8, kind="ExternalInput").ap()
    xv = nc.dram_tensor("xv", (T, E), BF16, kind="ExternalInput").ap()
    wvh = nc.dram_tensor("wvh", (E, H), F8, kind="ExternalInput").ap()
    wvl = nc.dram_tensor("wvl", (E, H), F8, kind="ExternalInput").ap()
    masks = nc.dram_tensor("masks", (P, 2 * BLK), BF16, kind="ExternalInput").ap()
    out = nc.dram_tensor("out", (T, H), F32, kind="ExternalOutput").ap()

    def ko(a):  # [K, X] dram -> [128, K/128, X] view
        return a.rearrange("(ko p) t -> p ko t", p=P)

    with _SplitDrainTC(nc) as tc:
        with (
            tc.tile_pool(name="wts", bufs=1) as wts_pool,
            tc.tile_pool(name="xblk", bufs=2) as xq_pool,
            tc.tile_pool(name="useg", bufs=1) as u_pool,
            tc.tile_pool(name="pt", bufs=1) as pt_pool,
            tc.tile_pool(name="wseg", bufs=1) as w_pool,
            tc.tile_pool(name="outs", bufs=3) as out_pool,
            tc.tile_pool(name="misc", bufs=1) as misc_pool,
            tc.tile_pool(name="ps_a", bufs=4, space="PSUM") as ps_a,
            tc.tile_pool(name="ps_o", bufs=3, space="PSUM") as ps_o,
            tc.tile_pool(name="ps_d", bufs=1, space="PSUM") as ps_d,
        ):
            # ---- resident weights / activations (DMA strictly in use-order:
            # M + xq block 0 first so PE starts ~7us in, then xk (S stage),
            # xv (w stage), wv by h-block (out stage)).
            mh_sb = wts_pool.tile([P, EO, E], F8, tag="mh")
            ml_sb = wts_pool.tile([P, EO, E], F8, tag="ml")
            xq0h_sb = xq_pool.tile([P, EO, BLK], F8, tag="xqh")
            xq0l_sb = xq_pool.tile([P, EO, BLK], F8, tag="xql")
            nc.sync.dma_start(xq0h_sb[:], ko(xqh)[:, :, 0:BLK])
            nc.sync.dma_start(mh_sb[:, :, 0 : E // 2], ko(mh)[:, :, 0 : E // 2])
            nc.sync.dma_start(mh_sb[:, :, E // 2 : E], ko(mh)[:, :, E // 2 : E])
            nc.sync.dma_start(xq0l_sb[:], ko(xql)[:, :, 0:BLK])
            nc.sync.dma_start(ml_sb[:, :, 0 : E // 2], ko(ml)[:, :, 0 : E // 2])
            nc.sync.dma_start(ml_sb[:, :, E // 2 : E], ko(ml)[:, :, E // 2 : E])
            masks_sb = misc_pool.tile([P, 2 * BLK], BF16, tag="masks")
            nc.sync.dma_start(masks_sb[:], masks)
            ones_sb = misc_pool.tile([P, P], BF16, tag="ones")
            nc.vector.memset(ones_sb[:], 1.0)
            bias_sb = misc_pool.tile([P, 1], F32, tag="bias")
            nc.vector.memset(bias_sb[:], EXP_BIAS)
            bias1_sb = misc_pool.tile([1, 1], F32, tag="bias1")
            nc.vector.memset(bias1_sb[:], EXP_BIAS)
            onesr_sb = misc_pool.tile([1, P], BF16, tag="onesr")
            nc.vector.memset(onesr_sb[:], 1.0)

            xkh_sb = wts_pool.tile([P, EO, T], F8, tag="xkh")
            xkl_sb = wts_pool.tile([P, EO, T], F8, tag="xkl")
            nc.sync.dma_start(xkh_sb[:], ko(xkh))
            nc.sync.dma_start(xkl_sb[:], ko(xkl))
            xv_sb = wts_pool.tile([P, TKT, E], BF16, tag="xv")
            for c in range(2):
                nc.sync.dma_start(
                    xv_sb[:, c * 8 : (c + 1) * 8, :],
                    ko(xv)[:, c * 8 : (c + 1) * 8, :],
                )
            wvh_sb = wts_pool.tile([P, EO, H], F8, tag="wvh")
            wvl_sb = wts_pool.tile([P, EO, H], F8, tag="wvl")
            for c in range(NBLK):
                nc.sync.dma_start(
                    wvh_sb[:, :, c * BLK : (c + 1) * BLK],
                    ko(wvh)[:, :, c * BLK : (c + 1) * BLK],
                )
                nc.sync.dma_start(
                    wvl_sb[:, :, c * BLK : (c + 1) * BLK],
                    ko(wvl)[:, :, c * BLK : (c + 1) * BLK],
                )

            def emit_u_stage(j, xqh_sb, xql_sb):
                # ---- u^T block [128, EO, 512] as fp8 hi/lo.  For j > 0 this
                # is emitted between w(j-1) and out(j-1): the u chains fill
                # the w-split PE bubble and out(j-1) covers the u-split tail.
                if xqh_sb is None:
                    xqh_sb = xq_pool.tile([P, EO, BLK], F8, tag="xqh")
                    xql_sb = xq_pool.tile([P, EO, BLK], F8, tag="xql")
                    nc.sync.dma_start(
                        xqh_sb[:], ko(xqh)[:, :, j * BLK : (j + 1) * BLK]
                    )
                    nc.sync.dma_start(
                        xql_sb[:], ko(xql)[:, :, j * BLK : (j + 1) * BLK]
                    )
                uh_sb = u_pool.tile([P, EO, BLK], F8, tag="uh", name=f"uh{j}")
                ul_sb = u_pool.tile([P, EO, BLK], F8, tag="ul", name=f"ul{j}")
                for eo in range(EO):
                    ps = ps_a.tile([P, BLK], F32, tag="ps_a")
                    n = 0
                    for a_, b_ in ((mh_sb, xqh_sb), (mh_sb, xql_sb), (ml_sb, xqh_sb)):
                        for c in range(EO // 2):
                            nc.tensor.matmul(
                                ps[:],
                                a_[:, 2 * c : 2 * c + 2, eo * P : (eo + 1) * P],
                                b_[:, 2 * c : 2 * c + 2, :],
                                start=(n == 0),
                                stop=(n == 3 * EO // 2 - 1),
                                perf_mode=DR,
                            )
                            n += 1
                    nc.scalar.copy(uh_sb[:, eo, :], ps[:])
                    nc.vector.tensor_tensor(
                        ul_sb[:, eo, :], ps[:], uh_sb[:, eo, :],
                        mybir.AluOpType.subtract,
                    )
                return uh_sb, ul_sb

            u_next = emit_u_stage(0, xq0h_sb, xq0l_sb)
            for j in range(NBLK):
                # The mask kidx <= qidx+1 needs tiles tk <= 4j+4, but tile
                # 4j+4 contains exactly ONE valid element (q = 512j+511,
                # k = 512j+512).  Handle it as a [1,1] scalar side-path and
                # run the dense loops over ntk = 4j+4 tiles only.
                has_sd = 4 * j + 4 < TKT
                ntk = 4 * j + 4 if has_sd else TKT
                uh_sb, ul_sb = u_next

                # ---- S^T tiles -> exp -> mask -> P^T [128, ntk, 512] bf16
                # den-row accumulates per tile: den[1, tq] += ones^T P^T
                pt_sb = pt_pool.tile([P, TKT, BLK], BF16)
                den_ps = ps_d.tile([P, BLK], F32)
                for t in range(ntk):
                    ps = ps_a.tile([P, BLK], F32, tag="ps_a")
                    # last e-pair last: covers the Act/DVE split tail of the
                    # final u chain (eo=6,7) with 9 instructions of work
                    n = 0
                    s_terms = ((xkh_sb, uh_sb), (xkl_sb, uh_sb), (xkh_sb, ul_sb))
                    for c in (0, 1, 2):
                        for a_, b_ in s_terms:
                            nc.tensor.matmul(
                                ps[:],
                                a_[:, 2 * c : 2 * c + 2, t * P : (t + 1) * P],
                                b_[:, 2 * c : 2 * c + 2, :],
                                start=(n == 0),
                                stop=False,
                                perf_mode=DR,
                            )
                            n += 1
                    for a_, b_ in s_terms:
                        nc.tensor.matmul(
                            ps[:],
                            a_[:, 6:8, t * P : (t + 1) * P],
                            b_[:, 6:8, :],
                            start=False,
                            stop=(n == 3 * EO // 2 - 1),
                            perf_mode=DR,
                        )
                        n += 1
                    nc.scalar.activation(
                        pt_sb[:, t, :],
                        ps[:],
                        mybir.ActivationFunctionType.Exp,
                        scale=EXP_SCALE,
                        bias=bias_sb[:],
                    )
                    m = t - 4 * j
                    if m >= 0:  # partial tile: zero the disallowed region
                        nc.vector.tensor_tensor(
                            pt_sb[:, t, :],
                            pt_sb[:, t, :],
                            masks_sb[:, BLK - m * P : 2 * BLK - m * P],
                            mybir.AluOpType.mult,
                        )
                    nc.tensor.matmul(
                        den_ps[:],
                        ones_sb[:],
                        pt_sb[:, t, :],
                        start=(t == 0),
                        stop=(not has_sd and t == ntk - 1),
                    )

                if has_sd:
                    # scalar S element: S[q0, k0], q0 = 512j+511, k0 = 512j+512
                    k0 = ntk * P  # column of k0 in the [E, T] layouts
                    sd_ps = ps_o.tile([1, 1], F32, tag="ps_o", name=f"sd_ps_{j}")
                    n = 0
                    for a_, b_ in ((xkh_sb, uh_sb), (xkl_sb, uh_sb), (xkh_sb, ul_sb)):
                        for c in range(EO // 2):
                            nc.tensor.matmul(
                                sd_ps[:],
                                a_[:, 2 * c : 2 * c + 2, k0 : k0 + 1],
                                b_[:, 2 * c : 2 * c + 2, BLK - 1 : BLK],
                                start=(n == 0),
                                stop=(n == 3 * EO // 2 - 1),
                                perf_mode=DR,
                            )
                            n += 1
                    pt1_sb = misc_pool.tile([1, 1], BF16, tag="pt1")
                    nc.scalar.activation(
                        pt1_sb[:],
                        sd_ps[:],
                        mybir.ActivationFunctionType.Exp,
                        scale=EXP_SCALE,
                        bias=bias1_sb[:],
                    )
                    nc.tensor.matmul(
                        den_ps[:, BLK - 1 : BLK],
                        onesr_sb[:],
                        pt1_sb[:],
                        start=False,
                        stop=True,
                        skip_group_check=True,
                    )

                # sigma[e, tq] = 1/den[tq], already replicated across
                # partitions because the den matmul used a full ones matrix.
                sg_sb = misc_pool.tile([P, BLK], F32, tag="sg")
                nc.vector.reciprocal(sg_sb[:], den_ps[:])

                # ---- w^T block: w^T = x_v^T P^T, normalized by sigma and
                # split to fp8 hi/lo for the DoubleRow out-stage.
                w_sb = w_pool.tile([P, EO, BLK], BF16, tag="w")
                wh_sb = w_pool.tile([P, EO, BLK], F8, tag="wh")
                wl_sb = w_pool.tile([P, EO, BLK], F8, tag="wl")
                for eo in range(EO):
                    ps = ps_o.tile([P, BLK], F32, tag="ps_o")
                    for t in range(ntk):
                        nc.tensor.matmul(
                            ps[:],
                            xv_sb[:, t, eo * P : (eo + 1) * P],
                            pt_sb[:, t, :],
                            start=(t == 0),
                            stop=(not has_sd and t == ntk - 1),
                        )
                    if has_sd:
                        nc.tensor.matmul(
                            ps[:, BLK - 1 : BLK],
                            xv_sb[0:1, ntk, eo * P : (eo + 1) * P],
                            pt1_sb[:],
                            start=False,
                            stop=True,
                            skip_group_check=True,
                        )
                    nc.vector.tensor_tensor(
                        w_sb[:, eo, :], ps[:], sg_sb[:], mybir.AluOpType.mult
                    )
                    nc.scalar.copy(wh_sb[:, eo, :], w_sb[:, eo, :])
                    nc.vector.tensor_tensor(
                        wl_sb[:, eo, :], w_sb[:, eo, :], wh_sb[:, eo, :],
                        mybir.AluOpType.subtract,
                    )

                if j + 1 < NBLK:
                    u_next = emit_u_stage(j + 1, None, None)

                # ---- out[tq, h] = (w_n (16 Wv)^T) / 16
                for hb in range(NBLK):
                    for s in range(NBLK):
                        o_ps = ps_o.tile(
                            [P, BLK], F32, tag="ps_o", name=f"o_ps_{j}_{hb}_{s}"
                        )
                        n = 0
                        o_terms = (
                            (wh_sb, wvh_sb), (wh_sb, wvl_sb), (wl_sb, wvh_sb)
                        )
                        for c in (0, 1, 2):
                            for a_, b_ in o_terms:
                                nc.tensor.matmul(
                                    o_ps[:],
                                    a_[:, 2 * c : 2 * c + 2, s * P : (s + 1) * P],
                                    b_[:, 2 * c : 2 * c + 2, hb * BLK : (hb + 1) * BLK],
                                    start=(n == 0),
                                    stop=False,
                                    perf_mode=DR,
                                )
                                n += 1
                        for a_, b_ in o_terms:
                            nc.tensor.matmul(
                                o_ps[:],
                                a_[:, 6:8, s * P : (s + 1) * P],
                                b_[:, 6:8, hb * BLK : (hb + 1) * BLK],
                                start=False,
                                stop=(n == 3 * EO // 2 - 1),
                                perf_mode=DR,
                            )
                            n += 1
                        o_sb = out_pool.tile([P, BLK], F32, tag="o")
                        nc.vector.tensor_scalar_mul(o_sb[:], o_ps[:], 1.0 / 16.0)
                        nc.sync.dma_start(
                            out[
                                j * BLK + s * P : j * BLK + (s + 1) * P,
                                hb * BLK : (hb + 1) * BLK,
                            ],
                            o_sb[:],
                        )
    return nc


def _split_waits(nc, limit=1):
    """This walrus build accepts only one sync-wait per TPB instruction.
    Move excess waits onto same-engine nops inserted just before the
    instruction (engine sequencers execute in order, so this is
    semantically identical)."""
    k = 0
    for f in nc.m.functions:
        for blk in f.blocks:
            new = []
            for inst in blk.instructions:
                si = inst.sync_info
                waits = list(si.on_wait) if si and si.on_wait else []
                if len(waits) > limit:
                    for w in waits[:-limit]:
                        nop = mybir.InstNoOp(name=f"wsplit-{k}", ins=[], outs=[])
                        k += 1
                        nop.engine = inst.engine
                        nop.sync_info = mybir.SyncInfo(on_wait=[w], on_update=[])
                        new.append(nop)
                    si.on_wait = waits[-limit:]
                new.append(inst)
            blk.instructions[:] = new
    return nc


_NC_CACHE = None


def _get_nc():
    global _NC_CACHE
    if _NC_CACHE is None:
        _NC_CACHE = _split_waits(_build())
    return _NC_CACHE


def _host_masks():
    # wide[p, c] = (p <= c - 511); slice [BLK-128m : 2*BLK-128m] yields the
    # partial-tile mask for diagonal offset m (p <= f - 128m + 1).
    p = np.arange(P)[:, None]
    c = np.arange(2 * BLK)[None, :]
    return (p <= c - (BLK - 1)).astype(ml_dtypes.bfloat16)


def _split8(x):
    f8 = ml_dtypes.float8_e4m3
    hi = np.ascontiguousarray(x).astype(f8)
    lo = (x - hi.astype(np.float32)).astype(f8)
    return hi, lo


def _prep_in_maps(key, query, value, Wk, Wq, Wv):
    bf = ml_dtypes.bfloat16
    M = (MSCALE * (Wq.astype(np.float32).T @ Wk.astype(np.float32))).astype(
        np.float32
    )  # [E, E]
    mh, ml = _split8(M)
    wvh, wvl = _split8((MSCALE * Wv.T).astype(np.float32))  # [E, H], 16x
    masks = _host_masks()

    in_maps = []
    for b in range(B):
        xqh, xql = _split8(query[b].T)
        xkh, xkl = _split8(key[b].T)
        in_maps.append(
            {
                "xqh": xqh, "xql": xql,
                "xkh": xkh, "xkl": xkl,
                "mh": mh, "ml": ml,
                "xv": value[b].astype(bf),
                "wvh": wvh, "wvl": wvl,
                "masks": masks,
            }
        )
    return in_maps


def kernel(key, query, value, Wk, Wq, Wv):
    in_maps = _prep_in_maps(key, query, value, Wk, Wq, Wv)
    nc = _get_nc()
    res = bass_utils.run_bass_kernel_spmd(nc, in_maps, core_ids=list(range(B)))
    return np.stack([res.results[i]["out"] for i in range(B)]).astype(np.float32)


# revision 24
# speedup vs baseline: 2.4665x; 1.0043x over previous
"""Trainium2 Bass kernel for a single attention head with input projections.

Per-batch-element (B=8 -> one NeuronCore each), algebraically reassociated:
  Since the head only uses q through S = q k^T, fold the two projections:
     M  = Wq^T Wk                  [E, E]   (host, shared across batch)
     u  = x_q M                    [T, E]   (instead of q = x_q Wq^T [T, H])
     S  = u x_k^T / sqrt(E)        [T, T]   (contract E=1024, not H=2048)
     P  = masked exp(S)            (kidx <= qidx + 1, one super-diagonal)
     w  = P x_v                    [T, E]   (contract T before Wv)
     out= (w Wv^T) / den           [T, H]
  This cuts per-core matmul work from ~49 GFLOP to ~24.5 GFLOP.

  The u and S GEMMs additionally run as fp8e4 (e4m3) DoubleRow matmuls
  (0.5 PE-cycles per output column per 256-deep contraction = 4x bf16
  throughput) using an error-compensated split: every operand x is held as
  x_hi = fp8(x), x_lo = fp8(x - x_hi), and x@y is computed as
  xh@yh + xl@yh + xh@yl (3 fp8 matmuls = 0.75x the bf16 cost, ~0.3% error).
  M is pre-scaled by 16 on the host so its entries sit in e4m3's normal
  range; the 1/16 is folded into the exp() input scale.  P^T, w and the
  final GEMM stay bf16 (exp spans too much dynamic range for fp8).

Layout: scores are computed TRANSPOSED (S^T tiles, tk on partitions) so
P^T feeds the w matmul directly and the softmax denominator is a
ones-matmul; w is produced as w^T [E, T] which is exactly the stationary
operand the final GEMM needs.  No on-chip transposes anywhere.
"""

import math
import sys

sys.path.insert(0, "/opt/trn_rl_repo")

import ml_dtypes
import numpy as np

import concourse.bass as bass
import concourse.mybir as mybir
import concourse.tile as tile
from concourse import bass_utils
from concourse.tile import ScopedClock

B, T, E, H = 8, 2048, 1024, 2048
P = 128
EO = E // P          # 8 e-subtiles
TKT = T // P         # 16 tk tiles
NBLK = 4             # tq blocks of 512
BLK = T // NBLK      # 512
F8 = mybir.dt.float8e4
BF16 = mybir.dt.bfloat16
F32 = mybir.dt.float32
DR = mybir.MatmulPerfMode.DoubleRow
MSCALE = 16.0                              # host scale on M (fp8 range)
EXP_SCALE = 1.0 / (MSCALE * math.sqrt(E))  # applied to S psum
EXP_BIAS = -7.0 * math.log(2.0)            # pt = exp(S/sqrt(E)) / 128


class _SplitDrainTC(tile.TileContext):
    """This walrus build rejects >1 sync-wait on the kernel-tail SP Drain
    ("Too many sync wait commands").  Spread the waits over preceding nops
    on the same engine instead — sequentially equivalent."""

    def _drain_and_barrier(self, tick_clock, wait_clock):
        nc = self.nc
        nops = [nc.sync.nop(nofuse=True) for _ in range(40)]
        drain_inst = nc.sync.drain()
        wait_clock.add_sem_waits(
            drain_inst.ins, ScopedClock({None: tick_clock.global_clock})
        )
        si = drain_inst.ins.sync_info
        waits = list(si.on_wait or [])
        if len(waits) > 1:
            assert len(waits) <= len(nops) + 1
            si.on_wait = [waits[-1]]
            for w, nop in zip(waits[:-1], nops):
                nsi = nop.ins.sync_info
                if nsi is None:
                    nop.ins.sync_info = mybir.SyncInfo(on_wait=[w], on_update=[])
                else:
                    nsi.on_wait = [w]
        nc.all_engine_barrier()
        popped = nc._tile_sem_poison_stack.pop()
        assert popped is self._sem_poison
        nc.clear_and_free_semaphores(list(self.sems.allocated().values()))
        nc.all_engine_barrier()


def _build():
    nc = bass.Bass("TRN2", target_bir_lowering=False, debug=False)

    xqh = nc.dram_tensor("xqh", (E, T), F8, kind="ExternalInput").ap()
    xql = nc.dram_tensor("xql", (E, T), F8, kind="ExternalInput").ap()
    xkh = nc.dram_tensor("xkh", (E, T), F8, kind="ExternalInput").ap()
    xkl = nc.dram_tensor("xkl", (E, T), F8, kind="ExternalInput").ap()
    mh = nc.dram_tensor("mh", (E, E), F8, kind="ExternalInput").ap()
    ml = nc.dram_tensor("ml", (E, E), F8, kind="ExternalInput").ap()
    xv = nc.dram_tensor("xv", (T, E), BF16, kind="ExternalInput").ap()
    wvh = nc.dram_tensor("wvh", (E, H), F8, kind="ExternalInput").ap()
    wvl = nc.dram_tensor("wvl", (E, H), F8, kind="ExternalInput").ap()
    masks = nc.dram_tensor("masks", (P, 2 * BLK), BF16, kind="ExternalInput").ap()
    out = nc.dram_tensor("out", (T, H), F32, kind="ExternalOutput").ap()

    def ko(a):  # [K, X] dram -> [128, K/128, X] view
        return a.rearrange("(ko p) t -> p ko t", p=P)

    with _SplitDrainTC(nc) as tc:
        with (
            tc.tile_pool(name="wts", bufs=1) as wts_pool,
            tc.tile_pool(name="xblk", bufs=2) as xq_pool,
            tc.tile_pool(name="useg", bufs=1) as u_pool,
            tc.tile_pool(name="pt", bufs=1) as pt_pool,
            tc.tile_pool(name="wseg", bufs=1) as w_pool,
            tc.tile_pool(name="outs", bufs=3) as out_pool,
            tc.tile_pool(name="misc", bufs=1) as misc_pool,
            tc.tile_pool(name="ps_a", bufs=3, space="PSUM") as ps_a,
            tc.tile_pool(name="ps_o", bufs=4, space="PSUM") as ps_o,
            tc.tile_pool(name="ps_d", bufs=1, space="PSUM") as ps_d,
        ):
            # ---- resident weights / activations (DMA strictly in use-order:
            # M + xq block 0 first so PE starts ~7us in, then xk (S stage),
            # xv (w stage), wv by h-block (out stage)).
            mh_sb = wts_pool.tile([P, EO, E], F8, tag="mh")
            ml_sb = wts_pool.tile([P, EO, E], F8, tag="ml")
            xq0h_sb = xq_pool.tile([P, EO, BLK], F8, tag="xqh")
            xq0l_sb = xq_pool.tile([P, EO, BLK], F8, tag="xql")
            nc.sync.dma_start(xq0h_sb[:], ko(xqh)[:, :, 0:BLK])
            nc.sync.dma_start(mh_sb[:, :, 0 : E // 2], ko(mh)[:, :, 0 : E // 2])
            nc.sync.dma_start(mh_sb[:, :, E // 2 : E], ko(mh)[:, :, E // 2 : E])
            nc.sync.dma_start(xq0l_sb[:], ko(xql)[:, :, 0:BLK])
            nc.sync.dma_start(ml_sb[:, :, 0 : E // 2], ko(ml)[:, :, 0 : E // 2])
            nc.sync.dma_start(ml_sb[:, :, E // 2 : E], ko(ml)[:, :, E // 2 : E])
            masks_sb = misc_pool.tile([P, 2 * BLK], BF16, tag="masks")
            nc.sync.dma_start(masks_sb[:], masks)
            ones_sb = misc_pool.tile([P, P], BF16, tag="ones")
            nc.vector.memset(ones_sb[:], 1.0)
            bias_sb = misc_pool.tile([P, 1], F32, tag="bias")
            nc.vector.memset(bias_sb[:], EXP_BIAS)
            bias1_sb = misc_pool.tile([1, 1], F32, tag="bias1")
            nc.vector.memset(bias1_sb[:], EXP_BIAS)
            onesr_sb = misc_pool.tile([1, P], BF16, tag="onesr")
            nc.vector.memset(onesr_sb[:], 1.0)

            # ---- PE warm-up: the first ~6us are DMA-bound; run throwaway
            # matmuls on memset data so the PE p-state is fully ramped (and
            # the pipeline full) when the real chains arrive.
            scratch_sb = misc_pool.tile([P, BLK], BF16, tag="scratch")
            nc.vector.memset(scratch_sb[:], 0.0)
            warm_ps = ps_a.tile([P, BLK], F32, tag="ps_a", name="warm")
            for n in range(22):
                nc.tensor.matmul(
                    warm_ps[:],
                    ones_sb[:],
                    scratch_sb[:],
                    start=(n == 0),
                    stop=(n == 21),
                )

            xkh_sb = wts_pool.tile([P, EO, T], F8, tag="xkh")
            xkl_sb = wts_pool.tile([P, EO, T], F8, tag="xkl")
            nc.sync.dma_start(xkh_sb[:], ko(xkh))
            nc.sync.dma_start(xkl_sb[:], ko(xkl))
            xv_sb = wts_pool.tile([P, TKT, E], BF16, tag="xv")
            for c in range(2):
                nc.sync.dma_start(
                    xv_sb[:, c * 8 : (c + 1) * 8, :],
                    ko(xv)[:, c * 8 : (c + 1) * 8, :],
                )
            wvh_sb = wts_pool.tile([P, EO, H], F8, tag="wvh")
            wvl_sb = wts_pool.tile([P, EO, H], F8, tag="wvl")
            for c in range(NBLK):
                nc.sync.dma_start(
                    wvh_sb[:, :, c * BLK : (c + 1) * BLK],
                    ko(wvh)[:, :, c * BLK : (c + 1) * BLK],
                )
                nc.sync.dma_start(
                    wvl_sb[:, :, c * BLK : (c + 1) * BLK],
                    ko(wvl)[:, :, c * BLK : (c + 1) * BLK],
                )

            def prefetch_xq(j):
                xqh_sb = xq_pool.tile([P, EO, BLK], F8, tag="xqh")
                xql_sb = xq_pool.tile([P, EO, BLK], F8, tag="xql")
                nc.sync.dma_start(
                    xqh_sb[:], ko(xqh)[:, :, j * BLK : (j + 1) * BLK]
                )
                nc.sync.dma_start(
                    xql_sb[:], ko(xql)[:, :, j * BLK : (j + 1) * BLK]
                )
                return xqh_sb, xql_sb

            def emit_u_stage(j, xq_tiles):
                # ---- u^T block [128, EO, 512] as fp8 hi/lo.  For j > 0 this
                # is emitted between w(j-1) and out(j-1): the u chains fill
                # the w-split PE bubble and out(j-1) covers the u-split tail.
                xqh_sb, xql_sb = xq_tiles
                uh_sb = u_pool.tile([P, EO, BLK], F8, tag="uh", name=f"uh{j}")
                ul_sb = u_pool.tile([P, EO, BLK], F8, tag="ul", name=f"ul{j}")
                for eo in range(EO):
                    ps = ps_a.tile([P, BLK], F32, tag="ps_a")
                    n = 0
                    for a_, b_ in ((mh_sb, xqh_sb), (mh_sb, xql_sb), (ml_sb, xqh_sb)):
                        for c in range(EO // 2):
                            nc.tensor.matmul(
                                ps[:],
                                a_[:, 2 * c : 2 * c + 2, eo * P : (eo + 1) * P],
                                b_[:, 2 * c : 2 * c + 2, :],
                                start=(n == 0),
                                stop=(n == 3 * EO // 2 - 1),
                                perf_mode=DR,
                            )
                            n += 1
                    nc.scalar.copy(uh_sb[:, eo, :], ps[:])
                    nc.vector.tensor_tensor(
                        ul_sb[:, eo, :], ps[:], uh_sb[:, eo, :],
                        mybir.AluOpType.subtract,
                    )
                return uh_sb, ul_sb

            u_next = emit_u_stage(0, (xq0h_sb, xq0l_sb))
            xq_next = prefetch_xq(1)
            for j in range(NBLK):
                # The mask kidx <= qidx+1 needs tiles tk <= 4j+4, but tile
                # 4j+4 contains exactly ONE valid element (q = 512j+511,
                # k = 512j+512).  Handle it as a [1,1] scalar side-path and
                # run the dense loops over ntk = 4j+4 tiles only.
                has_sd = 4 * j + 4 < TKT
                ntk = 4 * j + 4 if has_sd else TKT
                uh_sb, ul_sb = u_next

                # ---- S^T tiles -> exp -> mask -> P^T [128, ntk, 512] bf16
                # den-row accumulates per tile: den[1, tq] += ones^T P^T
                pt_sb = pt_pool.tile([P, TKT, BLK], BF16)
                den_ps = ps_d.tile([P, BLK], F32)
                for t in range(ntk):
                    ps = ps_a.tile([P, BLK], F32, tag="ps_a")
                    # last e-pair last: covers the Act/DVE split tail of the
                    # final u chain (eo=6,7) with 9 instructions of work
                    n = 0
                    s_terms = ((xkh_sb, uh_sb), (xkl_sb, uh_sb), (xkh_sb, ul_sb))
                    for c in (0, 1, 2):
                        for a_, b_ in s_terms:
                            nc.tensor.matmul(
                                ps[:],
                                a_[:, 2 * c : 2 * c + 2, t * P : (t + 1) * P],
                                b_[:, 2 * c : 2 * c + 2, :],
                                start=(n == 0),
                                stop=False,
                                perf_mode=DR,
                            )
                            n += 1
                    for a_, b_ in s_terms:
                        nc.tensor.matmul(
                            ps[:],
                            a_[:, 6:8, t * P : (t + 1) * P],
                            b_[:, 6:8, :],
                            start=False,
                            stop=(n == 3 * EO // 2 - 1),
                            perf_mode=DR,
                        )
                        n += 1
                    nc.scalar.activation(
                        pt_sb[:, t, :],
                        ps[:],
                        mybir.ActivationFunctionType.Exp,
                        scale=EXP_SCALE,
                        bias=bias_sb[:],
                    )
                    m = t - 4 * j
                    if m >= 0:  # partial tile: zero the disallowed region
                        nc.vector.tensor_tensor(
                            pt_sb[:, t, :],
                            pt_sb[:, t, :],
                            masks_sb[:, BLK - m * P : 2 * BLK - m * P],
                            mybir.AluOpType.mult,
                        )
                    nc.tensor.matmul(
                        den_ps[:],
                        ones_sb[:],
                        pt_sb[:, t, :],
                        start=(t == 0),
                        stop=(not has_sd and t == ntk - 1),
                    )

                if has_sd:
                    # scalar S element: S[q0, k0], q0 = 512j+511, k0 = 512j+512
                    k0 = ntk * P  # column of k0 in the [E, T] layouts
                    sd_ps = ps_o.tile([1, 1], F32, tag="ps_o", name=f"sd_ps_{j}")
                    n = 0
                    for a_, b_ in ((xkh_sb, uh_sb), (xkl_sb, uh_sb), (xkh_sb, ul_sb)):
                        for c in range(EO // 2):
                            nc.tensor.matmul(
                                sd_ps[:],
                                a_[:, 2 * c : 2 * c + 2, k0 : k0 + 1],
                                b_[:, 2 * c : 2 * c + 2, BLK - 1 : BLK],
                                start=(n == 0),
                                stop=(n == 3 * EO // 2 - 1),
                                perf_mode=DR,
                            )
                            n += 1
                    pt1_sb = misc_pool.tile([1, 1], BF16, tag="pt1")
                    nc.scalar.activation(
                        pt1_sb[:],
                        sd_ps[:],
                        mybir.ActivationFunctionType.Exp,
                        scale=EXP_SCALE,
                        bias=bias1_sb[:],
                    )
                    nc.tensor.matmul(
                        den_ps[:, BLK - 1 : BLK],
                        onesr_sb[:],
                        pt1_sb[:],
                        start=False,
                        stop=True,
                        skip_group_check=True,
                    )

                # sigma[e, tq] = 1/den[tq], already replicated across
                # partitions because the den matmul used a full ones matrix.
                sg_sb = misc_pool.tile([P, BLK], F32, tag="sg")
                nc.vector.reciprocal(sg_sb[:], den_ps[:])

                # ---- w^T block: w^T = x_v^T P^T, normalized by sigma and
                # split to fp8 hi/lo for the DoubleRow out-stage.
                w_sb = w_pool.tile([P, EO, BLK], BF16, tag="w")
                wh_sb = w_pool.tile([P, EO, BLK], F8, tag="wh")
                wl_sb = w_pool.tile([P, EO, BLK], F8, tag="wl")
                for eo in range(EO):
                    ps = ps_o.tile([P, BLK], F32, tag="ps_o")
                    for t in range(ntk):
                        nc.tensor.matmul(
                            ps[:],
                            xv_sb[:, t, eo * P : (eo + 1) * P],
                            pt_sb[:, t, :],
                            start=(t == 0),
                            stop=(not has_sd and t == ntk - 1),
                        )
                    if has_sd:
                        nc.tensor.matmul(
                            ps[:, BLK - 1 : BLK],
                            xv_sb[0:1, ntk, eo * P : (eo + 1) * P],
                            pt1_sb[:],
                            start=False,
                            stop=True,
                            skip_group_check=True,
                        )
                    nc.vector.tensor_tensor(
                        w_sb[:, eo, :], ps[:], sg_sb[:], mybir.AluOpType.mult
                    )
                    nc.scalar.copy(wh_sb[:, eo, :], w_sb[:, eo, :])
                    nc.vector.tensor_tensor(
                        wl_sb[:, eo, :], w_sb[:, eo, :], wh_sb[:, eo, :],
                        mybir.AluOpType.subtract,
                    )

                if j + 1 < NBLK:
                    u_next = emit_u_stage(j + 1, xq_next)
                    if j + 2 < NBLK:
                        xq_next = prefetch_xq(j + 2)

                # ---- out[tq, h] = (w_n (16 Wv)^T) / 16
                for hb in range(NBLK):
                    for s in range(NBLK):
                        o_ps = ps_o.tile(
                            [P, BLK], F32, tag="ps_o", name=f"o_ps_{j}_{hb}_{s}"
                        )
                        n = 0
                        o_terms = (
                            (wh_sb, wvh_sb), (wh_sb, wvl_sb), (wl_sb, wvh_sb)
                        )
                        for c in (0, 1, 2):
                            for a_, b_ in o_terms:
                                nc.tensor.matmul(
                                    o_ps[:],
                                    a_[:, 2 * c : 2 * c + 2, s * P : (s + 1) * P],
                                    b_[:, 2 * c : 2 * c + 2, hb * BLK : (hb + 1) * BLK],
                                    start=(n == 0),
                                    stop=False,
                                    perf_mode=DR,
                                )
                                n += 1
                        for a_, b_ in o_terms:
                            nc.tensor.matmul(
                                o_ps[:],
                                a_[:, 6:8, s * P : (s + 1) * P],
                                b_[:, 6:8, hb * BLK : (hb + 1) * BLK],
                                start=False,
                                stop=(n == 3 * EO // 2 - 1),
                                perf_mode=DR,
                            )
                            n += 1
                        last = j == NBLK - 1 and hb == NBLK - 1 and s == NBLK - 1
                        if not last:
                            o_sb = out_pool.tile([P, BLK], F32, tag="o")
                            nc.vector.tensor_scalar_mul(
                                o_sb[:], o_ps[:], 1.0 / 16.0
                            )
                            nc.sync.dma_start(
                                out[
                                    j * BLK + s * P : j * BLK + (s + 1) * P,
                                    hb * BLK : (hb + 1) * BLK,
                                ],
                                o_sb[:],
                            )
                        else:
                            # split the final tile so its copy and DMA
                            # pipeline during kernel wind-down
                            for hf in range(2):
                                o_sb = out_pool.tile(
                                    [P, BLK // 2], F32, tag="o2", name=f"o2_{hf}"
                                )
                                nc.vector.tensor_scalar_mul(
                                    o_sb[:],
                                    o_ps[:, hf * (BLK // 2) : (hf + 1) * (BLK // 2)],
                                    1.0 / 16.0,
                                )
                                nc.sync.dma_start(
                                    out[
                                        j * BLK + s * P : j * BLK + (s + 1) * P,
                                        hb * BLK + hf * (BLK // 2) : hb * BLK
                                        + (hf + 1) * (BLK // 2),
                                    ],
                                    o_sb[:],
                                )
    return nc


def _split_waits(nc, limit=1):
    """This walrus build accepts only one sync-wait per TPB instruction.
    Move excess waits onto same-engine nops inserted just before the
    instruction (engine sequencers execute in order, so this is
    semantically identical)."""
    k = 0
    for f in nc.m.functions:
        for blk in f.blocks:
            new = []
            for inst in blk.instructions:
                si = inst.sync_info
                waits = list(si.on_wait) if si and si.on_wait else []
                if len(waits) > limit:
                    for w in waits[:-limit]:
                        nop = mybir.InstNoOp(name=f"wsplit-{k}", ins=[], outs=[])
                        k += 1
                        nop.engine = inst.engine
                        nop.sync_info = mybir.SyncInfo(on_wait=[w], on_update=[])
                        new.append(nop)
                    si.on_wait = waits[-limit:]
                new.append(inst)
            blk.instructions[:] = new
    return nc


_NC_CACHE = None


def _get_nc():
    global _NC_CACHE
    if _NC_CACHE is None:
        _NC_CACHE = _split_waits(_build())
    return _NC_CACHE


def _host_masks():
    # wide[p, c] = (p <= c - 511); slice [BLK-128m : 2*BLK-128m] yields the
    # partial-tile mask for diagonal offset m (p <= f - 128m + 1).
    p = np.arange(P)[:, None]
    c = np.arange(2 * BLK)[None, :]
    return (p <= c - (BLK - 1)).astype(ml_dtypes.bfloat16)


def _split8(x):
    f8 = ml_dtypes.float8_e4m3
    hi = np.ascontiguousarray(x).astype(f8)
    lo = (x - hi.astype(np.float32)).astype(f8)
    return hi, lo


def _prep_in_maps(key, query, value, Wk, Wq, Wv):
    bf = ml_dtypes.bfloat16
    M = (MSCALE * (Wq.astype(np.float32).T @ Wk.astype(np.float32))).astype(
        np.float32
    )  # [E, E]
    mh, ml = _split8(M)
    wvh, wvl = _split8((MSCALE * Wv.T).astype(np.float32))  # [E, H], 16x
    masks = _host_masks()

    in_maps = []
    for b in range(B):
        xqh, xql = _split8(query[b].T)
        xkh, xkl = _split8(key[b].T)
        in_maps.append(
            {
                "xqh": xqh, "xql": xql,
                "xkh": xkh, "xkl": xkl,
                "mh": mh, "ml": ml,
                "xv": value[b].astype(bf),
                "wvh": wvh, "wvl": wvl,
                "masks": masks,
            }
        )
    return in_maps


def kernel(key, query, value, Wk, Wq, Wv):
    in_maps = _prep_in_maps(key, query, value, Wk, Wq, Wv)
    nc = _get_nc()
    res = bass_utils.run_bass_kernel_spmd(nc, in_maps, core_ids=list(range(B)))
    return np.stack([res.results[i]["out"] for i in range(B)]).astype(np.float32)


# revision 32
# speedup vs baseline: 2.4742x; 1.0031x over previous
"""Trainium2 Bass kernel for a single attention head with input projections.

Per-batch-element (B=8 -> one NeuronCore each), algebraically reassociated:
  Since the head only uses q through S = q k^T, fold the two projections:
     M  = Wq^T Wk                  [E, E]   (host, shared across batch)
     u  = x_q M                    [T, E]   (instead of q = x_q Wq^T [T, H])
     S  = u x_k^T / sqrt(E)        [T, T]   (contract E=1024, not H=2048)
     P  = masked exp(S)            (kidx <= qidx + 1, one super-diagonal)
     w  = P x_v                    [T, E]   (contract T before Wv)
     out= (w Wv^T) / den           [T, H]
  This cuts per-core matmul work from ~49 GFLOP to ~24.5 GFLOP.

  The u, S and out GEMMs run as fp8e4 (e4m3) DoubleRow matmuls (0.5
  PE-cycles per output column per 256-deep contraction = 4x bf16
  throughput) using an error-compensated split: every operand x is held as
  x_hi = fp8(x), x_lo = fp8(x - x_hi), and x@y is computed as
  xh@yh + xl@yh + xh@yl (3 fp8 matmuls = 0.75x the bf16 cost, ~0.3% error).
  M and Wv are pre-scaled by 16 on the host so their entries sit in e4m3's
  normal range (the inverses are folded into the exp() input scale and the
  output copy).  P^T and the w matmul stay bf16: exp spans too much
  dynamic range for fp8, and normalizing P^T first would serialize behind
  the full denominator.  w IS normalizable (den known by then): it is
  scaled by sigma = 1/den, split to fp8, and feeds the fp8 out GEMM.

Layout: scores are computed TRANSPOSED (S^T tiles, tk on partitions) so
P^T feeds the w matmul directly and the softmax denominator is a
ones-matmul; w is produced as w^T [E, T] which is exactly the stationary
operand the final GEMM needs.  No on-chip transposes anywhere.  The
denominator matmul uses a full ones[128,128] stationary so den lands
replicated across partitions and reciprocal() directly yields the sigma
broadcast matrix.  Each tq-block's masked score region includes one tile
(tk = 4j+4) with exactly ONE valid element (q = 512j+511, k = 512j+512);
it is handled as a [1,1] scalar side-chain instead of full S/w tiles.
u(j+1) is emitted between w(j) and out(j) so its chains fill the
w-split pipeline bubble.
"""

import math
import sys

sys.path.insert(0, "/opt/trn_rl_repo")

import ml_dtypes
import numpy as np

import concourse.bass as bass
import concourse.mybir as mybir
import concourse.tile as tile
from concourse import bass_utils
from concourse.tile import ScopedClock

B, T, E, H = 8, 2048, 1024, 2048
P = 128
EO = E // P          # 8 e-subtiles
TKT = T // P         # 16 tk tiles
NBLK = 4             # tq blocks of 512
BLK = T // NBLK      # 512
F8 = mybir.dt.float8e4
BF16 = mybir.dt.bfloat16
F32 = mybir.dt.float32
DR = mybir.MatmulPerfMode.DoubleRow
MSCALE = 16.0                              # host scale on M (fp8 range)
EXP_SCALE = 1.0 / (MSCALE * math.sqrt(E))  # applied to S psum
EXP_BIAS = -7.0 * math.log(2.0)            # pt = exp(S/sqrt(E)) / 128


class _SplitDrainTC(tile.TileContext):
    """This walrus build rejects >1 sync-wait on the kernel-tail SP Drain
    ("Too many sync wait commands").  Spread the waits over preceding nops
    on the same engine instead — sequentially equivalent."""

    def _drain_and_barrier(self, tick_clock, wait_clock):
        nc = self.nc
        nops = [nc.sync.nop(nofuse=True) for _ in range(40)]
        drain_inst = nc.sync.drain()
        wait_clock.add_sem_waits(
            drain_inst.ins, ScopedClock({None: tick_clock.global_clock})
        )
        si = drain_inst.ins.sync_info
        waits = list(si.on_wait or [])
        if len(waits) > 1:
            assert len(waits) <= len(nops) + 1
            si.on_wait = [waits[-1]]
            for w, nop in zip(waits[:-1], nops):
                nsi = nop.ins.sync_info
                if nsi is None:
                    nop.ins.sync_info = mybir.SyncInfo(on_wait=[w], on_update=[])
                else:
                    nsi.on_wait = [w]
        nc.all_engine_barrier()
        popped = nc._tile_sem_poison_stack.pop()
        assert popped is self._sem_poison
        nc.clear_and_free_semaphores(list(self.sems.allocated().values()))
        nc.all_engine_barrier()


def _build():
    nc = bass.Bass("TRN2", target_bir_lowering=False, debug=False)

    xqh = nc.dram_tensor("xqh", (E, T), F8, kind="ExternalInput").ap()
    xql = nc.dram_tensor("xql", (E, T), F8, kind="ExternalInput").ap()
    xkh = nc.dram_tensor("xkh", (E, T), F8, kind="ExternalInput").ap()
    xkl = nc.dram_tensor("xkl", (E, T), F8, kind="ExternalInput").ap()
    mh = nc.dram_tensor("mh", (E, E), F8, kind="ExternalInput").ap()
    ml = nc.dram_tensor("ml", (E, E), F8, kind="ExternalInput").ap()
    xv = nc.dram_tensor("xv", (T, E), BF16, kind="ExternalInput").ap()
    wvh = nc.dram_tensor("wvh", (E, H), F8, kind="ExternalInput").ap()
    wvl = nc.dram_tensor("wvl", (E, H), F8, kind="ExternalInput").ap()
    masks = nc.dram_tensor("masks", (P, 2 * BLK), BF16, kind="ExternalInput").ap()
    out = nc.dram_tensor("out", (T, H), F32, kind="ExternalOutput").ap()

    def ko(a):  # [K, X] dram -> [128, K/128, X] view
        return a.rearrange("(ko p) t -> p ko t", p=P)

    with _SplitDrainTC(nc) as tc:
        with (
            tc.tile_pool(name="wts", bufs=1) as wts_pool,
            tc.tile_pool(name="xblk", bufs=2) as xq_pool,
            tc.tile_pool(name="useg", bufs=1) as u_pool,
            tc.tile_pool(name="pt", bufs=1) as pt_pool,
            tc.tile_pool(name="wseg", bufs=1) as w_pool,
            tc.tile_pool(name="outs", bufs=3) as out_pool,
            tc.tile_pool(name="misc", bufs=1) as misc_pool,
            tc.tile_pool(name="ps_a", bufs=3, space="PSUM") as ps_a,
            tc.tile_pool(name="ps_o", bufs=4, space="PSUM") as ps_o,
            tc.tile_pool(name="ps_d", bufs=1, space="PSUM") as ps_d,
        ):
            # ---- resident weights / activations (DMA strictly in use-order:
            # M + xq block 0 first so PE starts ~7us in, then xk (S stage),
            # xv (w stage), wv by h-block (out stage)).
            mh_sb = wts_pool.tile([P, EO, E], F8, tag="mh")
            ml_sb = wts_pool.tile([P, EO, E], F8, tag="ml")
            xq0h_sb = xq_pool.tile([P, EO, BLK], F8, tag="xqh")
            xq0l_sb = xq_pool.tile([P, EO, BLK], F8, tag="xql")
            nc.sync.dma_start(xq0h_sb[:], ko(xqh)[:, :, 0:BLK])
            nc.sync.dma_start(mh_sb[:, :, 0 : E // 2], ko(mh)[:, :, 0 : E // 2])
            nc.sync.dma_start(mh_sb[:, :, E // 2 : E], ko(mh)[:, :, E // 2 : E])
            nc.sync.dma_start(xq0l_sb[:], ko(xql)[:, :, 0:BLK])
            nc.sync.dma_start(ml_sb[:, :, 0 : E // 2], ko(ml)[:, :, 0 : E // 2])
            nc.sync.dma_start(ml_sb[:, :, E // 2 : E], ko(ml)[:, :, E // 2 : E])
            masks_sb = misc_pool.tile([P, 2 * BLK], BF16, tag="masks")
            nc.sync.dma_start(masks_sb[:], masks)
            ones_sb = misc_pool.tile([P, P], BF16, tag="ones")
            nc.vector.memset(ones_sb[:], 1.0)
            bias_sb = misc_pool.tile([P, 1], F32, tag="bias")
            nc.vector.memset(bias_sb[:], EXP_BIAS)
            bias1_sb = misc_pool.tile([1, 1], F32, tag="bias1")
            nc.vector.memset(bias1_sb[:], EXP_BIAS)
            onesr_sb = misc_pool.tile([1, P], BF16, tag="onesr")
            nc.vector.memset(onesr_sb[:], 1.0)

            # ---- PE warm-up: the first ~6us are DMA-bound; run throwaway
            # matmuls on memset data so the PE p-state is fully ramped (and
            # the pipeline full) when the real chains arrive.
            scratch_sb = misc_pool.tile([P, BLK], BF16, tag="scratch")
            nc.vector.memset(scratch_sb[:], 0.0)
            warm_ps = ps_a.tile([P, BLK], F32, tag="ps_a", name="warm")
            for n in range(22):
                nc.tensor.matmul(
                    warm_ps[:],
                    ones_sb[:],
                    scratch_sb[:],
                    start=(n == 0),
                    stop=(n == 21),
                )

            xkh_sb = wts_pool.tile([P, EO, T], F8, tag="xkh")
            xkl_sb = wts_pool.tile([P, EO, T], F8, tag="xkl")
            nc.sync.dma_start(xkh_sb[:], ko(xkh))
            nc.sync.dma_start(xkl_sb[:], ko(xkl))
            xv_sb = wts_pool.tile([P, TKT, E], BF16, tag="xv")
            for c in range(2):
                nc.sync.dma_start(
                    xv_sb[:, c * 8 : (c + 1) * 8, :],
                    ko(xv)[:, c * 8 : (c + 1) * 8, :],
                )
            wvh_sb = wts_pool.tile([P, EO, H], F8, tag="wvh")
            wvl_sb = wts_pool.tile([P, EO, H], F8, tag="wvl")
            for c in range(NBLK):
                nc.sync.dma_start(
                    wvh_sb[:, :, c * BLK : (c + 1) * BLK],
                    ko(wvh)[:, :, c * BLK : (c + 1) * BLK],
                )
                nc.sync.dma_start(
                    wvl_sb[:, :, c * BLK : (c + 1) * BLK],
                    ko(wvl)[:, :, c * BLK : (c + 1) * BLK],
                )

            def prefetch_xq(j):
                xqh_sb = xq_pool.tile([P, EO, BLK], F8, tag="xqh")
                xql_sb = xq_pool.tile([P, EO, BLK], F8, tag="xql")
                nc.sync.dma_start(
                    xqh_sb[:], ko(xqh)[:, :, j * BLK : (j + 1) * BLK]
                )
                nc.sync.dma_start(
                    xql_sb[:], ko(xql)[:, :, j * BLK : (j + 1) * BLK]
                )
                return xqh_sb, xql_sb

            def emit_u_stage(j, xq_tiles):
                # ---- u^T block [128, EO, 512] as fp8 hi/lo.  For j > 0 this
                # is emitted between w(j-1) and out(j-1): the u chains fill
                # the w-split PE bubble and out(j-1) covers the u-split tail.
                xqh_sb, xql_sb = xq_tiles
                uh_sb = u_pool.tile([P, EO, BLK], F8, tag="uh", name=f"uh{j}")
                ul_sb = u_pool.tile([P, EO, BLK], F8, tag="ul", name=f"ul{j}")
                for eo in range(EO):
                    ps = ps_a.tile([P, BLK], F32, tag="ps_a")
                    n = 0
                    for a_, b_ in ((mh_sb, xqh_sb), (mh_sb, xql_sb), (ml_sb, xqh_sb)):
                        for c in range(EO // 2):
                            nc.tensor.matmul(
                                ps[:],
                                a_[:, 2 * c : 2 * c + 2, eo * P : (eo + 1) * P],
                                b_[:, 2 * c : 2 * c + 2, :],
                                start=(n == 0),
                                stop=(n == 3 * EO // 2 - 1),
                                perf_mode=DR,
                            )
                            n += 1
                    nc.scalar.copy(uh_sb[:, eo, :], ps[:])
                    nc.vector.tensor_tensor(
                        ul_sb[:, eo, :], ps[:], uh_sb[:, eo, :],
                        mybir.AluOpType.subtract,
                    )
                return uh_sb, ul_sb

            u_next = emit_u_stage(0, (xq0h_sb, xq0l_sb))
            xq_next = prefetch_xq(1)
            for j in range(NBLK):
                # The mask kidx <= qidx+1 needs tiles tk <= 4j+4, but tile
                # 4j+4 contains exactly ONE valid element (q = 512j+511,
                # k = 512j+512).  Handle it as a [1,1] scalar side-path and
                # run the dense loops over ntk = 4j+4 tiles only.
                has_sd = 4 * j + 4 < TKT
                ntk = 4 * j + 4 if has_sd else TKT
                uh_sb, ul_sb = u_next

                # ---- S^T tiles -> exp -> mask -> P^T [128, ntk, 512] bf16
                # den-row accumulates per tile: den[1, tq] += ones^T P^T
                pt_sb = pt_pool.tile([P, TKT, BLK], BF16)
                den_ps = ps_d.tile([P, BLK], F32)
                for t in range(ntk):
                    ps = ps_a.tile([P, BLK], F32, tag="ps_a")
                    # last e-pair last: covers the Act/DVE split tail of the
                    # final u chain (eo=6,7) with 9 instructions of work
                    n = 0
                    s_terms = ((xkh_sb, uh_sb), (xkl_sb, uh_sb), (xkh_sb, ul_sb))
                    for c in (0, 1, 2):
                        for a_, b_ in s_terms:
                            nc.tensor.matmul(
                                ps[:],
                                a_[:, 2 * c : 2 * c + 2, t * P : (t + 1) * P],
                                b_[:, 2 * c : 2 * c + 2, :],
                                start=(n == 0),
                                stop=False,
                                perf_mode=DR,
                            )
                            n += 1
                    for a_, b_ in s_terms:
                        nc.tensor.matmul(
                            ps[:],
                            a_[:, 6:8, t * P : (t + 1) * P],
                            b_[:, 6:8, :],
                            start=False,
                            stop=(n == 3 * EO // 2 - 1),
                            perf_mode=DR,
                        )
                        n += 1
                    nc.scalar.activation(
                        pt_sb[:, t, :],
                        ps[:],
                        mybir.ActivationFunctionType.Exp,
                        scale=EXP_SCALE,
                        bias=bias_sb[:],
                    )
                    m = t - 4 * j
                    if m >= 0:  # partial tile: zero the disallowed region
                        nc.vector.tensor_tensor(
                            pt_sb[:, t, :],
                            pt_sb[:, t, :],
                            masks_sb[:, BLK - m * P : 2 * BLK - m * P],
                            mybir.AluOpType.mult,
                        )
                    nc.tensor.matmul(
                        den_ps[:],
                        ones_sb[:],
                        pt_sb[:, t, :],
                        start=(t == 0),
                        stop=(not has_sd and t == ntk - 1),
                    )

                if has_sd:
                    # scalar S element: S[q0, k0], q0 = 512j+511, k0 = 512j+512
                    k0 = ntk * P  # column of k0 in the [E, T] layouts
                    sd_ps = ps_o.tile([1, 1], F32, tag="ps_o", name=f"sd_ps_{j}")
                    n = 0
                    for a_, b_ in ((xkh_sb, uh_sb), (xkl_sb, uh_sb), (xkh_sb, ul_sb)):
                        for c in range(EO // 2):
                            nc.tensor.matmul(
                                sd_ps[:],
                                a_[:, 2 * c : 2 * c + 2, k0 : k0 + 1],
                                b_[:, 2 * c : 2 * c + 2, BLK - 1 : BLK],
                                start=(n == 0),
                                stop=(n == 3 * EO // 2 - 1),
                                perf_mode=DR,
                            )
                            n += 1
                    pt1_sb = misc_pool.tile([1, 1], BF16, tag="pt1")
                    nc.scalar.activation(
                        pt1_sb[:],
                        sd_ps[:],
                        mybir.ActivationFunctionType.Exp,
                        scale=EXP_SCALE,
                        bias=bias1_sb[:],
                    )
                    nc.tensor.matmul(
                        den_ps[:, BLK - 1 : BLK],
                        onesr_sb[:],
                        pt1_sb[:],
                        start=False,
                        stop=True,
                        skip_group_check=True,
                    )

                # sigma[e, tq] = 1/den[tq], already replicated across
                # partitions because the den matmul used a full ones matrix.
                sg_sb = misc_pool.tile([P, BLK], F32, tag="sg")
                nc.vector.reciprocal(sg_sb[:], den_ps[:])

                # ---- w^T block: w^T = x_v^T P^T, normalized by sigma and
                # split to fp8 hi/lo for the DoubleRow out-stage.
                w_sb = w_pool.tile([P, EO, BLK], BF16, tag="w")
                wh_sb = w_pool.tile([P, EO, BLK], F8, tag="wh")
                wl_sb = w_pool.tile([P, EO, BLK], F8, tag="wl")
                for eo in range(EO):
                    ps = ps_o.tile([P, BLK], F32, tag="ps_o")
                    for t in range(ntk):
                        nc.tensor.matmul(
                            ps[:],
                            xv_sb[:, t, eo * P : (eo + 1) * P],
                            pt_sb[:, t, :],
                            start=(t == 0),
                            stop=(not has_sd and t == ntk - 1),
                        )
                    if has_sd:
                        nc.tensor.matmul(
                            ps[:, BLK - 1 : BLK],
                            xv_sb[0:1, ntk, eo * P : (eo + 1) * P],
                            pt1_sb[:],
                            start=False,
                            stop=True,
                            skip_group_check=True,
                        )
                    nc.vector.tensor_tensor(
                        w_sb[:, eo, :], ps[:], sg_sb[:], mybir.AluOpType.mult
                    )
                    nc.scalar.copy(wh_sb[:, eo, :], w_sb[:, eo, :])
                    nc.vector.tensor_tensor(
                        wl_sb[:, eo, :], w_sb[:, eo, :], wh_sb[:, eo, :],
                        mybir.AluOpType.subtract,
                    )

                if j + 1 < NBLK:
                    u_next = emit_u_stage(j + 1, xq_next)
                    if j + 2 < NBLK:
                        xq_next = prefetch_xq(j + 2)

                # ---- out[tq, h] = (w_n (16 Wv)^T) / 16
                for hb in range(NBLK):
                    for s in range(NBLK):
                        o_ps = ps_o.tile(
                            [P, BLK], F32, tag="ps_o", name=f"o_ps_{j}_{hb}_{s}"
                        )
                        n = 0
                        o_terms = (
                            (wh_sb, wvh_sb), (wh_sb, wvl_sb), (wl_sb, wvh_sb)
                        )
                        for c in (0, 1, 2):
                            for a_, b_ in o_terms:
                                nc.tensor.matmul(
                                    o_ps[:],
                                    a_[:, 2 * c : 2 * c + 2, s * P : (s + 1) * P],
                                    b_[:, 2 * c : 2 * c + 2, hb * BLK : (hb + 1) * BLK],
                                    start=(n == 0),
                                    stop=False,
                                    perf_mode=DR,
                                )
                                n += 1
                        for a_, b_ in o_terms:
                            nc.tensor.matmul(
                                o_ps[:],
                                a_[:, 6:8, s * P : (s + 1) * P],
                                b_[:, 6:8, hb * BLK : (hb + 1) * BLK],
                                start=False,
                                stop=(n == 3 * EO // 2 - 1),
                                perf_mode=DR,
                            )
                            n += 1
                        last = j == NBLK - 1 and hb == NBLK - 1 and s == NBLK - 1
                        if not last:
                            o_sb = out_pool.tile([P, BLK], F32, tag="o")
                            nc.vector.tensor_scalar_mul(
                                o_sb[:], o_ps[:], 1.0 / 16.0
                            )
                            nc.sync.dma_start(
                                out[
                                    j * BLK + s * P : j * BLK + (s + 1) * P,
                                    hb * BLK : (hb + 1) * BLK,
                                ],
                                o_sb[:],
                            )
                        else:
                            # split the final tile so its copy and DMA
                            # pipeline during kernel wind-down
                            for hf in range(2):
                                o_sb = out_pool.tile(
                                    [P, BLK // 2], F32, tag="o2", name=f"o2_{hf}"
                                )
                                nc.vector.tensor_scalar_mul(
                                    o_sb[:],
                                    o_ps[:, hf * (BLK // 2) : (hf + 1) * (BLK // 2)],
                                    1.0 / 16.0,
                                )
                                nc.sync.dma_start(
                                    out[
                                        j * BLK + s * P : j * BLK + (s + 1) * P,
                                        hb * BLK + hf * (BLK // 2) : hb * BLK
                                        + (hf + 1) * (BLK // 2),
                                    ],
                                    o_sb[:],
                                )
    return nc


def _split_waits(nc, limit=1):
    """This walrus build accepts only one sync-wait per TPB instruction.
    Move excess waits onto same-engine nops inserted just before the
    instruction (engine sequencers execute in order, so this is
    semantically identical)."""
    k = 0
    for f in nc.m.functions:
        for blk in f.blocks:
            new = []
            for inst in blk.instructions:
                si = inst.sync_info
                waits = list(si.on_wait) if si and si.on_wait else []
                if len(waits) > limit:
                    for w in waits[:-limit]:
                        nop = mybir.InstNoOp(name=f"wsplit-{k}", ins=[], outs=[])
                        k += 1
                        nop.engine = inst.engine
                        nop.sync_info = mybir.SyncInfo(on_wait=[w], on_update=[])
                        new.append(nop)
                    si.on_wait = waits[-limit:]
                new.append(inst)
            blk.instructions[:] = new
    return nc


_NC_CACHE = None


def _get_nc():
    global _NC_CACHE
    if _NC_CACHE is None:
        _NC_CACHE = _split_waits(_build())
    return _NC_CACHE


def _host_masks():
    # wide[p, c] = (p <= c - 511); slice [BLK-128m : 2*BLK-128m] yields the
    # partial-tile mask for diagonal offset m (p <= f - 128m + 1).
    p = np.arange(P)[:, None]
    c = np.arange(2 * BLK)[None, :]
    return (p <= c - (BLK - 1)).astype(ml_dtypes.bfloat16)


def _split8(x):
    f8 = ml_dtypes.float8_e4m3
    hi = np.ascontiguousarray(x).astype(f8)
    lo = (x - hi.astype(np.float32)).astype(f8)
    return hi, lo


def _prep_in_maps(key, query, value, Wk, Wq, Wv):
    bf = ml_dtypes.bfloat16
    M = (MSCALE * (Wq.astype(np.float32).T @ Wk.astype(np.float32))).astype(
        np.float32
    )  # [E, E]
    mh, ml = _split8(M)
    wvh, wvl = _split8((MSCALE * Wv.T).astype(np.float32))  # [E, H], 16x
    masks = _host_masks()

    in_maps = []
    for b in range(B):
        xqh, xql = _split8(query[b].T)
        xkh, xkl = _split8(key[b].T)
        in_maps.append(
            {
                "xqh": xqh, "xql": xql,
                "xkh": xkh, "xkl": xkl,
                "mh": mh, "ml": ml,
                "xv": value[b].astype(bf),
                "wvh": wvh, "wvl": wvl,
                "masks": masks,
            }
        )
    return in_maps


def kernel(key, query, value, Wk, Wq, Wv):
    in_maps = _prep_in_maps(key, query, value, Wk, Wq, Wv)
    nc = _get_nc()
    res = bass_utils.run_bass_kernel_spmd(nc, in_maps, core_ids=list(range(B)))
    return np.stack([res.results[i]["out"] for i in range(B)]).astype(np.float32)


# revision 38
# speedup vs baseline: 2.4827x; 1.0035x over previous
"""Trainium2 Bass kernel for a single attention head with input projections.

Per-batch-element (B=8 -> one NeuronCore each), algebraically reassociated:
  Since the head only uses q through S = q k^T, fold the two projections:
     M  = Wq^T Wk                  [E, E]   (host, shared across batch)
     u  = x_q M                    [T, E]   (instead of q = x_q Wq^T [T, H])
     S  = u x_k^T / sqrt(E)        [T, T]   (contract E=1024, not H=2048)
     P  = masked exp(S)            (kidx <= qidx + 1, one super-diagonal)
     w  = P x_v                    [T, E]   (contract T before Wv)
     out= (w Wv^T) / den           [T, H]
  This cuts per-core matmul work from ~49 GFLOP to ~24.5 GFLOP.

  The u, S and out GEMMs run as fp8e4 (e4m3) DoubleRow matmuls (0.5
  PE-cycles per output column per 256-deep contraction = 4x bf16
  throughput) using an error-compensated split: every operand x is held as
  x_hi = fp8(x), x_lo = fp8(x - x_hi), and x@y is computed as
  xh@yh + xl@yh + xh@yl (3 fp8 matmuls = 0.75x the bf16 cost, ~0.3% error).
  M and Wv are pre-scaled by 16 on the host so their entries sit in e4m3's
  normal range (the inverses are folded into the exp() input scale and the
  output copy).  P^T and the w matmul stay bf16: exp spans too much
  dynamic range for fp8, and normalizing P^T first would serialize behind
  the full denominator.  w IS normalizable (den known by then): it is
  scaled by sigma = 1/den, split to fp8, and feeds the fp8 out GEMM.

Layout: scores are computed TRANSPOSED (S^T tiles, tk on partitions) so
P^T feeds the w matmul directly and the softmax denominator is a
ones-matmul; w is produced as w^T [E, T] which is exactly the stationary
operand the final GEMM needs.  No on-chip transposes anywhere.  The
denominator matmul uses a full ones[128,128] stationary so den lands
replicated across partitions and reciprocal() directly yields the sigma
broadcast matrix.  Each tq-block's masked score region includes one tile
(tk = 4j+4) with exactly ONE valid element (q = 512j+511, k = 512j+512);
it is handled as a [1,1] scalar side-chain instead of full S/w tiles.
u(j+1) is emitted between w(j) and out(j) so its chains fill the
w-split pipeline bubble.
"""

import math
import sys

sys.path.insert(0, "/opt/trn_rl_repo")

import ml_dtypes
import numpy as np

import concourse.bass as bass
import concourse.mybir as mybir
import concourse.tile as tile
from concourse import bass_utils
from concourse.tile import ScopedClock

B, T, E, H = 8, 2048, 1024, 2048
P = 128
EO = E // P          # 8 e-subtiles
TKT = T // P         # 16 tk tiles
NBLK = 4             # tq blocks of 512
BLK = T // NBLK      # 512
F8 = mybir.dt.float8e4
BF16 = mybir.dt.bfloat16
F32 = mybir.dt.float32
DR = mybir.MatmulPerfMode.DoubleRow
MSCALE = 16.0                              # host scale on M (fp8 range)
EXP_SCALE = 1.0 / (MSCALE * math.sqrt(E))  # applied to S psum
EXP_BIAS = -7.0 * math.log(2.0)            # pt = exp(S/sqrt(E)) / 128


class _SplitDrainTC(tile.TileContext):
    """This walrus build rejects >1 sync-wait on the kernel-tail SP Drain
    ("Too many sync wait commands").  Spread the waits over preceding nops
    on the same engine instead — sequentially equivalent."""

    def _drain_and_barrier(self, tick_clock, wait_clock):
        nc = self.nc
        nops = [nc.sync.nop(nofuse=True) for _ in range(40)]
        drain_inst = nc.sync.drain()
        wait_clock.add_sem_waits(
            drain_inst.ins, ScopedClock({None: tick_clock.global_clock})
        )
        si = drain_inst.ins.sync_info
        waits = list(si.on_wait or [])
        if len(waits) > 1:
            assert len(waits) <= len(nops) + 1
            si.on_wait = [waits[-1]]
            for w, nop in zip(waits[:-1], nops):
                nsi = nop.ins.sync_info
                if nsi is None:
                    nop.ins.sync_info = mybir.SyncInfo(on_wait=[w], on_update=[])
                else:
                    nsi.on_wait = [w]
        nc.all_engine_barrier()
        popped = nc._tile_sem_poison_stack.pop()
        assert popped is self._sem_poison
        nc.clear_and_free_semaphores(list(self.sems.allocated().values()))
        nc.all_engine_barrier()


def _build():
    nc = bass.Bass("TRN2", target_bir_lowering=False, debug=False)

    xqh = nc.dram_tensor("xqh", (E, T), F8, kind="ExternalInput").ap()
    xql = nc.dram_tensor("xql", (E, T), F8, kind="ExternalInput").ap()
    xkh = nc.dram_tensor("xkh", (E, T), F8, kind="ExternalInput").ap()
    xkl = nc.dram_tensor("xkl", (E, T), F8, kind="ExternalInput").ap()
    mh = nc.dram_tensor("mh", (E, E), F8, kind="ExternalInput").ap()
    ml = nc.dram_tensor("ml", (E, E), F8, kind="ExternalInput").ap()
    xv = nc.dram_tensor("xv", (T, E), BF16, kind="ExternalInput").ap()
    wvh = nc.dram_tensor("wvh", (E, H), F8, kind="ExternalInput").ap()
    wvl = nc.dram_tensor("wvl", (E, H), F8, kind="ExternalInput").ap()
    masks = nc.dram_tensor("masks", (P, 2 * BLK), BF16, kind="ExternalInput").ap()
    out = nc.dram_tensor("out", (T, H), F32, kind="ExternalOutput").ap()

    def ko(a):  # [K, X] dram -> [128, K/128, X] view
        return a.rearrange("(ko p) t -> p ko t", p=P)

    with _SplitDrainTC(nc) as tc:
        with (
            tc.tile_pool(name="wts", bufs=1) as wts_pool,
            tc.tile_pool(name="xblk", bufs=2) as xq_pool,
            tc.tile_pool(name="useg", bufs=1) as u_pool,
            tc.tile_pool(name="pt", bufs=1) as pt_pool,
            tc.tile_pool(name="wseg", bufs=1) as w_pool,
            tc.tile_pool(name="outs", bufs=4) as out_pool,
            tc.tile_pool(name="misc", bufs=1) as misc_pool,
            tc.tile_pool(name="ps_a", bufs=3, space="PSUM") as ps_a,
            tc.tile_pool(name="ps_o", bufs=4, space="PSUM") as ps_o,
            tc.tile_pool(name="ps_d", bufs=1, space="PSUM") as ps_d,
        ):
            # ---- resident weights / activations (DMA strictly in use-order:
            # M + xq block 0 first so PE starts ~7us in, then xk (S stage),
            # xv (w stage), wv by h-block (out stage)).
            mh_sb = wts_pool.tile([P, EO, E], F8, tag="mh")
            ml_sb = wts_pool.tile([P, EO, E], F8, tag="ml")
            xq0h_sb = xq_pool.tile([P, EO, BLK], F8, tag="xqh")
            xq0l_sb = xq_pool.tile([P, EO, BLK], F8, tag="xql")
            nc.sync.dma_start(xq0h_sb[:], ko(xqh)[:, :, 0:BLK])
            nc.sync.dma_start(mh_sb[:, :, 0 : E // 2], ko(mh)[:, :, 0 : E // 2])
            nc.sync.dma_start(mh_sb[:, :, E // 2 : E], ko(mh)[:, :, E // 2 : E])
            nc.sync.dma_start(xq0l_sb[:], ko(xql)[:, :, 0:BLK])
            nc.sync.dma_start(ml_sb[:, :, 0 : E // 2], ko(ml)[:, :, 0 : E // 2])
            nc.sync.dma_start(ml_sb[:, :, E // 2 : E], ko(ml)[:, :, E // 2 : E])
            masks_sb = misc_pool.tile([P, 2 * BLK], BF16, tag="masks")
            nc.sync.dma_start(masks_sb[:], masks)
            ones_sb = misc_pool.tile([P, P], BF16, tag="ones")
            nc.vector.memset(ones_sb[:], 1.0)
            bias_sb = misc_pool.tile([P, 1], F32, tag="bias")
            nc.vector.memset(bias_sb[:], EXP_BIAS)
            bias1_sb = misc_pool.tile([1, 1], F32, tag="bias1")
            nc.vector.memset(bias1_sb[:], EXP_BIAS)
            onesr_sb = misc_pool.tile([1, P], BF16, tag="onesr")
            nc.vector.memset(onesr_sb[:], 1.0)

            # ---- PE warm-up: the first ~6us are DMA-bound; run throwaway
            # matmuls on memset data so the PE p-state is fully ramped (and
            # the pipeline full) when the real chains arrive.
            scratch_sb = misc_pool.tile([P, BLK], BF16, tag="scratch")
            nc.vector.memset(scratch_sb[:], 0.0)
            warm_ps = ps_a.tile([P, BLK], F32, tag="ps_a", name="warm")
            for n in range(22):
                nc.tensor.matmul(
                    warm_ps[:],
                    ones_sb[:],
                    scratch_sb[:],
                    start=(n == 0),
                    stop=(n == 21),
                )

            # interleave the remaining resident loads in first-use order:
            # xk cols for S(0..1), xv tiles for w(0), wv h-block for out(0),
            # then the rest round-robin ahead of their consumers.
            xkh_sb = wts_pool.tile([P, EO, T], F8, tag="xkh")
            xkl_sb = wts_pool.tile([P, EO, T], F8, tag="xkl")
            xv_sb = wts_pool.tile([P, TKT, E], BF16, tag="xv")
            wvh_sb = wts_pool.tile([P, EO, H], F8, tag="wvh")
            wvl_sb = wts_pool.tile([P, EO, H], F8, tag="wvl")

            def ld_xk(c0, c1):
                sl = slice(c0 * P, c1 * P)
                nc.sync.dma_start(xkh_sb[:, :, sl], ko(xkh)[:, :, sl])
                nc.sync.dma_start(xkl_sb[:, :, sl], ko(xkl)[:, :, sl])

            def ld_xv(t0, t1):
                nc.sync.dma_start(
                    xv_sb[:, t0:t1, :], ko(xv)[:, t0:t1, :]
                )

            def ld_wv(c):
                sl = slice(c * BLK, (c + 1) * BLK)
                nc.sync.dma_start(wvh_sb[:, :, sl], ko(wvh)[:, :, sl])
                nc.sync.dma_start(wvl_sb[:, :, sl], ko(wvl)[:, :, sl])

            ld_xk(0, 8)      # S(0) needs tiles 0..4, S(1) through 8
            ld_xv(0, 5)      # w(0) needs tiles 0..4
            ld_wv(0)         # out(0) hb=0
            ld_xk(8, 16)
            ld_xv(5, 9)      # w(1)
            ld_wv(1)
            ld_xv(9, 16)
            ld_wv(2)
            ld_wv(3)

            def prefetch_xq(j):
                xqh_sb = xq_pool.tile([P, EO, BLK], F8, tag="xqh")
                xql_sb = xq_pool.tile([P, EO, BLK], F8, tag="xql")
                nc.sync.dma_start(
                    xqh_sb[:], ko(xqh)[:, :, j * BLK : (j + 1) * BLK]
                )
                nc.sync.dma_start(
                    xql_sb[:], ko(xql)[:, :, j * BLK : (j + 1) * BLK]
                )
                return xqh_sb, xql_sb

            def emit_u_stage(j, xq_tiles):
                # ---- u^T block [128, EO, 512] as fp8 hi/lo.  For j > 0 this
                # is emitted between w(j-1) and out(j-1): the u chains fill
                # the w-split PE bubble and out(j-1) covers the u-split tail.
                xqh_sb, xql_sb = xq_tiles
                uh_sb = u_pool.tile([P, EO, BLK], F8, tag="uh", name=f"uh{j}")
                ul_sb = u_pool.tile([P, EO, BLK], F8, tag="ul", name=f"ul{j}")
                for eo in range(EO):
                    ps = ps_a.tile([P, BLK], F32, tag="ps_a")
                    n = 0
                    for a_, b_ in ((mh_sb, xqh_sb), (mh_sb, xql_sb), (ml_sb, xqh_sb)):
                        for c in range(EO // 2):
                            nc.tensor.matmul(
                                ps[:],
                                a_[:, 2 * c : 2 * c + 2, eo * P : (eo + 1) * P],
                                b_[:, 2 * c : 2 * c + 2, :],
                                start=(n == 0),
                                stop=(n == 3 * EO // 2 - 1),
                                perf_mode=DR,
                            )
                            n += 1
                    nc.scalar.copy(uh_sb[:, eo, :], ps[:])
                    nc.vector.tensor_tensor(
                        ul_sb[:, eo, :], ps[:], uh_sb[:, eo, :],
                        mybir.AluOpType.subtract,
                    )
                return uh_sb, ul_sb

            u_next = emit_u_stage(0, (xq0h_sb, xq0l_sb))
            xq_next = prefetch_xq(1)
            for j in range(NBLK):
                # The mask kidx <= qidx+1 needs tiles tk <= 4j+4, but tile
                # 4j+4 contains exactly ONE valid element (q = 512j+511,
                # k = 512j+512).  Handle it as a [1,1] scalar side-path and
                # run the dense loops over ntk = 4j+4 tiles only.
                has_sd = 4 * j + 4 < TKT
                ntk = 4 * j + 4 if has_sd else TKT
                uh_sb, ul_sb = u_next

                # ---- S^T tiles -> exp -> mask -> P^T [128, ntk, 512] bf16
                # den-row accumulates per tile: den[1, tq] += ones^T P^T
                pt_sb = pt_pool.tile([P, TKT, BLK], BF16)
                den_ps = ps_d.tile([P, BLK], F32)
                for t in range(ntk):
                    ps = ps_a.tile([P, BLK], F32, tag="ps_a")
                    # last e-pair last: covers the Act/DVE split tail of the
                    # final u chain (eo=6,7) with 9 instructions of work
                    n = 0
                    s_terms = ((xkh_sb, uh_sb), (xkl_sb, uh_sb), (xkh_sb, ul_sb))
                    for c in (0, 1, 2):
                        for a_, b_ in s_terms:
                            nc.tensor.matmul(
                                ps[:],
                                a_[:, 2 * c : 2 * c + 2, t * P : (t + 1) * P],
                                b_[:, 2 * c : 2 * c + 2, :],
                                start=(n == 0),
                                stop=False,
                                perf_mode=DR,
                            )
                            n += 1
                    for a_, b_ in s_terms:
                        nc.tensor.matmul(
                            ps[:],
                            a_[:, 6:8, t * P : (t + 1) * P],
                            b_[:, 6:8, :],
                            start=False,
                            stop=(n == 3 * EO // 2 - 1),
                            perf_mode=DR,
                        )
                        n += 1
                    nc.scalar.activation(
                        pt_sb[:, t, :],
                        ps[:],
                        mybir.ActivationFunctionType.Exp,
                        scale=EXP_SCALE,
                        bias=bias_sb[:],
                    )
                    m = t - 4 * j
                    if m >= 0:  # partial tile: zero the disallowed region
                        nc.vector.tensor_tensor(
                            pt_sb[:, t, :],
                            pt_sb[:, t, :],
                            masks_sb[:, BLK - m * P : 2 * BLK - m * P],
                            mybir.AluOpType.mult,
                        )
                    nc.tensor.matmul(
                        den_ps[:],
                        ones_sb[:],
                        pt_sb[:, t, :],
                        start=(t == 0),
                        stop=(not has_sd and t == ntk - 1),
                    )

                if has_sd:
                    # scalar S element: S[q0, k0], q0 = 512j+511, k0 = 512j+512
                    k0 = ntk * P  # column of k0 in the [E, T] layouts
                    sd_ps = ps_o.tile([1, 1], F32, tag="ps_o", name=f"sd_ps_{j}")
                    n = 0
                    for a_, b_ in ((xkh_sb, uh_sb), (xkl_sb, uh_sb), (xkh_sb, ul_sb)):
                        for c in range(EO // 2):
                            nc.tensor.matmul(
                                sd_ps[:],
                                a_[:, 2 * c : 2 * c + 2, k0 : k0 + 1],
                                b_[:, 2 * c : 2 * c + 2, BLK - 1 : BLK],
                                start=(n == 0),
                                stop=(n == 3 * EO // 2 - 1),
                                perf_mode=DR,
                            )
                            n += 1
                    pt1_sb = misc_pool.tile([1, 1], BF16, tag="pt1")
                    nc.scalar.activation(
                        pt1_sb[:],
                        sd_ps[:],
                        mybir.ActivationFunctionType.Exp,
                        scale=EXP_SCALE,
                        bias=bias1_sb[:],
                    )
                    nc.tensor.matmul(
                        den_ps[:, BLK - 1 : BLK],
                        onesr_sb[:],
                        pt1_sb[:],
                        start=False,
                        stop=True,
                        skip_group_check=True,
                    )

                # sigma[e, tq] = 1/den[tq], already replicated across
                # partitions because the den matmul used a full ones matrix.
                sg_sb = misc_pool.tile([P, BLK], F32, tag="sg")
                nc.vector.reciprocal(sg_sb[:], den_ps[:])

                # ---- w^T block: w^T = x_v^T P^T, normalized by sigma and
                # split to fp8 hi/lo for the DoubleRow out-stage.
                w_sb = w_pool.tile([P, EO, BLK], BF16, tag="w")
                wh_sb = w_pool.tile([P, EO, BLK], F8, tag="wh")
                wl_sb = w_pool.tile([P, EO, BLK], F8, tag="wl")
                for eo in range(EO):
                    ps = ps_o.tile([P, BLK], F32, tag="ps_o")
                    for t in range(ntk):
                        nc.tensor.matmul(
                            ps[:],
                            xv_sb[:, t, eo * P : (eo + 1) * P],
                            pt_sb[:, t, :],
                            start=(t == 0),
                            stop=(not has_sd and t == ntk - 1),
                        )
                    if has_sd:
                        nc.tensor.matmul(
                            ps[:, BLK - 1 : BLK],
                            xv_sb[0:1, ntk, eo * P : (eo + 1) * P],
                            pt1_sb[:],
                            start=False,
                            stop=True,
                            skip_group_check=True,
                        )
                    nc.vector.tensor_tensor(
                        w_sb[:, eo, :], ps[:], sg_sb[:], mybir.AluOpType.mult
                    )
                    nc.scalar.copy(wh_sb[:, eo, :], w_sb[:, eo, :])
                    nc.vector.tensor_tensor(
                        wl_sb[:, eo, :], w_sb[:, eo, :], wh_sb[:, eo, :],
                        mybir.AluOpType.subtract,
                    )

                if j + 1 < NBLK:
                    u_next = emit_u_stage(j + 1, xq_next)
                    if j + 2 < NBLK:
                        xq_next = prefetch_xq(j + 2)

                # ---- out[tq, h] = (w_n (16 Wv)^T) / 16
                for hb in range(NBLK):
                    for s in range(NBLK):
                        o_ps = ps_o.tile(
                            [P, BLK], F32, tag="ps_o", name=f"o_ps_{j}_{hb}_{s}"
                        )
                        n = 0
                        o_terms = (
                            (wh_sb, wvh_sb), (wh_sb, wvl_sb), (wl_sb, wvh_sb)
                        )
                        for c in (0, 1, 2):
                            for a_, b_ in o_terms:
                                nc.tensor.matmul(
                                    o_ps[:],
                                    a_[:, 2 * c : 2 * c + 2, s * P : (s + 1) * P],
                                    b_[:, 2 * c : 2 * c + 2, hb * BLK : (hb + 1) * BLK],
                                    start=(n == 0),
                                    stop=False,
                                    perf_mode=DR,
                                )
                                n += 1
                        for a_, b_ in o_terms:
                            nc.tensor.matmul(
                                o_ps[:],
                                a_[:, 6:8, s * P : (s + 1) * P],
                                b_[:, 6:8, hb * BLK : (hb + 1) * BLK],
                                start=False,
                                stop=(n == 3 * EO // 2 - 1),
                                perf_mode=DR,
                            )
                            n += 1
                        last = j == NBLK - 1 and hb == NBLK - 1 and s == NBLK - 1
                        if not last:
                            o_sb = out_pool.tile([P, BLK], F32, tag="o")
                            nc.vector.tensor_scalar_mul(
                                o_sb[:], o_ps[:], 1.0 / 16.0
                            )
                            nc.sync.dma_start(
                                out[
                                    j * BLK + s * P : j * BLK + (s + 1) * P,
                                    hb * BLK : (hb + 1) * BLK,
                                ],
                                o_sb[:],
                            )
                        else:
                            # split the final tile so its copy and DMA
                            # pipeline during kernel wind-down
                            for hf in range(2):
                                o_sb = out_pool.tile(
                                    [P, BLK // 2], F32, tag="o2", name=f"o2_{hf}"
                                )
                                nc.vector.tensor_scalar_mul(
                                    o_sb[:],
                                    o_ps[:, hf * (BLK // 2) : (hf + 1) * (BLK // 2)],
                                    1.0 / 16.0,
                                )
                                nc.sync.dma_start(
                                    out[
                                        j * BLK + s * P : j * BLK + (s + 1) * P,
                                        hb * BLK + hf * (BLK // 2) : hb * BLK
                                        + (hf + 1) * (BLK // 2),
                                    ],
                                    o_sb[:],
                                )
    return nc


def _split_waits(nc, limit=1):
    """This walrus build accepts only one sync-wait per TPB instruction.
    Move excess waits onto same-engine nops inserted just before the
    instruction (engine sequencers execute in order, so this is
    semantically identical)."""
    k = 0
    for f in nc.m.functions:
        for blk in f.blocks:
            new = []
            for inst in blk.instructions:
                si = inst.sync_info
                waits = list(si.on_wait) if si and si.on_wait else []
                if len(waits) > limit:
                    for w in waits[:-limit]:
                        nop = mybir.InstNoOp(name=f"wsplit-{k}", ins=[], outs=[])
                        k += 1
                        nop.engine = inst.engine
                        nop.sync_info = mybir.SyncInfo(on_wait=[w], on_update=[])
                        new.append(nop)
                    si.on_wait = waits[-limit:]
                new.append(inst)
            blk.instructions[:] = new
    return nc


_NC_CACHE = None


def _get_nc():
    global _NC_CACHE
    if _NC_CACHE is None:
        _NC_CACHE = _split_waits(_build())
    return _NC_CACHE


def _host_masks():
    # wide[p, c] = (p <= c - 511); slice [BLK-128m : 2*BLK-128m] yields the
    # partial-tile mask for diagonal offset m (p <= f - 128m + 1).
    p = np.arange(P)[:, None]
    c = np.arange(2 * BLK)[None, :]
    return (p <= c - (BLK - 1)).astype(ml_dtypes.bfloat16)


def _split8(x):
    f8 = ml_dtypes.float8_e4m3
    hi = np.ascontiguousarray(x).astype(f8)
    lo = (x - hi.astype(np.float32)).astype(f8)
    return hi, lo


def _prep_in_maps(key, query, value, Wk, Wq, Wv):
    bf = ml_dtypes.bfloat16
    M = (MSCALE * (Wq.astype(np.float32).T @ Wk.astype(np.float32))).astype(
        np.float32
    )  # [E, E]
    mh, ml = _split8(M)
    wvh, wvl = _split8((MSCALE * Wv.T).astype(np.float32))  # [E, H], 16x
    masks = _host_masks()

    in_maps = []
    for b in range(B):
        xqh, xql = _split8(query[b].T)
        xkh, xkl = _split8(key[b].T)
        in_maps.append(
            {
                "xqh": xqh, "xql": xql,
                "xkh": xkh, "xkl": xkl,
                "mh": mh, "ml": ml,
                "xv": value[b].astype(bf),
                "wvh": wvh, "wvl": wvl,
                "masks": masks,
            }
        )
    return in_maps


def kernel(key, query, value, Wk, Wq, Wv):
    in_maps = _prep_in_maps(key, query, value, Wk, Wq, Wv)
    nc = _get_nc()
    res = bass_utils.run_bass_kernel_spmd(nc, in_maps, core_ids=list(range(B)))
    return np.stack([res.results[i]["out"] for i in range(B)]).astype(np.float32)
